# revision 10
# baseline (speedup 1.0000x reference)
"""Trainium2 Bass kernel for nn_Graph_module_net_0_loss_type_18631568130084.

GNN message-passing block:
  gts       = relu(gt_feat @ Wg + bg)
  attn[i,j] = sigmoid(x[j]@Wq + x[i]@Wk + b_att)          (H == 1)
  atten     = (attn * (mr1+mr2) * col + f_diag) / CHILDS  ([B,H,Nj,Ni])
  o1 = relu(gconv1(x^T)); o1 += ln1(o1 @ atten)^T
  o2 = relu(gconv2(o1));  node_feat = ln2(o2 @ atten);  output2 = (o2 + node_feat^T)^T

Sharding: data-parallel over batch B=16 -> 2 batches per core on 8 cores.

v2 layout/dataflow notes (vs the v1 baseline at 180us):
 * ONE mask tensor: host pre-folds (m1+m2)*score*col into msT fp16 and folds
   the f_diag term exactly onto the diagonal as f[j]/sigmoid(l_jj), so the
   device computes atten^T = sigmoid_tile * msT_tile in a single gpsimd
   tensor_tensor per j-tile (no SWDGE accumulate, no per-tile diag fixup).
 * gts is computed in [OUT, N] layout so bg becomes a per-partition bias,
   fused into the ACT relu; the host un-transposes after gather.
 * LayerNorm rstd: variances of 4 i-tiles are packed into one [128,4] tile,
   one ACT Sqrt (bias=eps) + one DVE reciprocal per wave.  Batch order keeps
   ACT on the sigmoid table set for all sigmoids first, then the sqrt set:
   exactly 2 ACT table loads per program.
 * Engine balance: ACT = sigmoid + relu(+bias) + sqrt; DVE = PSUM-side ops
   (bn_stats/aggr, LN apply, relu o2, PSUM->SBUF copies); GPSIMD = SBUF-side
   ops (mask mult, LN outer = g*rstd, residual adds, output adds).
 * Few, large DMAs (inputs ~6 triggers, masks 16, outputs 3) instead of 102;
   outputs in fp16 (host casts back to fp32).
 * Stage D matmuls are issued jc-outer in waves of 4 i-tiles so PE can start
   contracting as soon as At[jc=0] exists; the 1/CHILDS scale cancels in the
   layernorms (eps rescaled by CHILDS^2).
 * The top-k "col" mask is computed exactly on the host: a cheap sufficient
   condition proves col == all-ones; otherwise an exact numpy replica runs.
"""

import numpy as np

B = 16
N = 1024
CIN = 256
MID = 512
OUT = 256
G = 4
CHILDS = 512
NCORES = 8
B_LOC = B // NCORES  # 2
NT = N // 128  # 8
EPS_LN = 1e-6 * float(CHILDS) ** 2  # eps rescaled because we drop the 1/CHILDS

F16 = np.float16
F32 = np.float32

_PROGRAM_CACHE = {}


def _build_program(beta1_nz: bool, beta2_nz: bool):
    import concourse.bacc as bacc
    import concourse.tile as tile
    from concourse import mybir

    f16 = mybir.dt.float16
    f32 = mybir.dt.float32
    AF = mybir.ActivationFunctionType
    OP = mybir.AluOpType

    nc = bacc.Bacc("TRN2", debug=False)

    def din(name, shape, dt):
        return nc.dram_tensor(name, shape, dt, kind="ExternalInput").ap()

    def dout(name, shape, dt):
        return nc.dram_tensor(name, shape, dt, kind="ExternalOutput").ap()

    # Per-core inputs (leading dim B_LOC where batch-dependent).
    msT_d = din("msT", [B_LOC, N, N], f16)       # (m1+m2)*score*col (+diag) ^T
    xT_d = din("xT", [B_LOC, CIN, N], f16)       # x^T   [c, n]
    gtT_d = din("gtT", [B_LOC, CIN, N], f16)     # gt^T  [c, n]
    lirow_d = din("lirow", [B_LOC, N], f16)      # x@Wk + b_att      (per-i row)
    ljT_d = din("ljT", [B_LOC, 128, NT], f32)    # x@Wq chunked      (per-j bias)
    # Replicated weights.
    wg_d = din("wgK", [2, 128, OUT], f16)        # Wg   (c-chunks)
    w1_d = din("w1K", [2, 128, MID], f16)        # block-diag W1^T (c-chunks)
    w2_d = din("w2K", [4, 128, OUT], f16)        # block-diag W2^T (m-chunks)
    bgcol_d = din("bgcol", [2, 128], f32)        # bg per o-tile (per-partition)
    b1_d = din("b1row", [1, MID], f16)
    b2_d = din("b2row", [1, OUT], f16)
    g1_d = din("g1row", [1, MID], f32)
    g2_d = din("g2row", [1, OUT], f32)
    beta1_d = din("beta1row", [1, MID], f32)
    beta2_d = din("beta2row", [1, OUT], f32)
    ident_d = din("ident", [128, 128], f16)
    ones_d = din("onescol", [1, 128], f16)

    gtsT_d = dout("gtsT", [B_LOC, OUT, N], f16)  # [o, n] - host un-transposes
    node_d = dout("node", [B_LOC, N, OUT], f16)
    out2_d = dout("out2", [B_LOC, N, OUT], f16)

    with tile.TileContext(nc) as tc:
        with tc.tile_pool(name="const", bufs=1) as constp, \
             tc.tile_pool(name="inp", bufs=1) as inp, \
             tc.tile_pool(name="at", bufs=1) as atp, \
             tc.tile_pool(name="big", bufs=1) as bigp, \
             tc.tile_pool(name="work", bufs=4) as workp, \
             tc.tile_pool(name="sg", bufs=3) as sgp, \
             tc.tile_pool(name="outs", bufs=1) as outp, \
             tc.tile_pool(name="mm", bufs=6, space="PSUM") as mmp, \
             tc.tile_pool(name="tp", bufs=2, space="PSUM") as tpp:

            # ---- constants (vector queue: idle at start, keeps sync free
            # for the mask/input DMAs) ----
            ident_t = constp.tile([128, 128], f16)
            nc.scalar.dma_start(out=ident_t, in_=ident_d)
            ones_t = constp.tile([1, 128], f16)
            nc.scalar.dma_start(out=ones_t, in_=ones_d)
            wg_t = constp.tile([128, 2, OUT], f16)
            nc.scalar.dma_start(out=wg_t, in_=wg_d.rearrange("c p f -> p c f"))
            w1_t = constp.tile([128, 2, MID], f16)
            nc.scalar.dma_start(out=w1_t, in_=w1_d.rearrange("c p f -> p c f"))
            w2_t = constp.tile([128, 4, OUT], f16)
            nc.scalar.dma_start(out=w2_t, in_=w2_d.rearrange("c p f -> p c f"))
            bgcol_t = constp.tile([128, 2], f32)
            nc.scalar.dma_start(out=bgcol_t, in_=bgcol_d.rearrange("o p -> p o"))
            b1_t = constp.tile([1, MID], f16)
            nc.scalar.dma_start(out=b1_t, in_=b1_d)
            b2_t = constp.tile([1, OUT], f16)
            nc.scalar.dma_start(out=b2_t, in_=b2_d)
            g1row_t = constp.tile([128, MID], f32)
            nc.scalar.dma_start(out=g1row_t, in_=g1_d.to_broadcast([128, MID]))
            g2row_t = constp.tile([128, OUT], f32)
            nc.scalar.dma_start(out=g2row_t, in_=g2_d.to_broadcast([128, OUT]))
            if beta1_nz:
                beta1_t = constp.tile([128, MID], f32)
                nc.scalar.dma_start(out=beta1_t, in_=beta1_d.to_broadcast([128, MID]))
            if beta2_nz:
                beta2_t = constp.tile([128, OUT], f32)
                nc.scalar.dma_start(out=beta2_t, in_=beta2_d.to_broadcast([128, OUT]))
            eps_t = constp.tile([128, 1], f32)
            nc.vector.memset(eps_t, EPS_LN)

            # ---- bulk input DMAs (sync queue), masks per (b, jt) tile so
            # downstream matmuls unblock tile-by-tile ----
            xT_t = inp.tile([128, B_LOC, 2, N], f16)
            nc.sync.dma_start(
                out=xT_t, in_=xT_d.rearrange("b (c p) n -> p b c n", p=128)
            )
            gtT_t = inp.tile([128, B_LOC, 2, N], f16)
            nc.sync.dma_start(
                out=gtT_t, in_=gtT_d.rearrange("b (c p) n -> p b c n", p=128)
            )
            lirow_t = inp.tile([128, B_LOC, N], f16)
            nc.sync.dma_start(
                out=lirow_t,
                in_=lirow_d[None].to_broadcast([128, B_LOC, N]),
            )
            ljT_t = inp.tile([128, B_LOC, NT], f32)
            nc.sync.dma_start(out=ljT_t, in_=ljT_d.rearrange("b p t -> p b t"))

            At = [
                [atp.tile([128, N], f16, name=f"At{b}_{jt}", tag=f"At{b}_{jt}") for jt in range(NT)]
                for b in range(B_LOC)
            ]
            for b in range(B_LOC):
                for jt in range(NT):
                    nc.sync.dma_start(
                        out=At[b][jt], in_=msT_d[b, jt * 128 : (jt + 1) * 128, :]
                    )

            # Per-batch activation tensors (both batches resident).
            o1t = [bigp.tile([128, NT, MID], f16, name=f"o1t{b}", tag=f"o1t{b}") for b in range(B_LOC)]
            o1nT = [bigp.tile([128, NT, MID], f16, name=f"o1nT{b}", tag=f"o1nT{b}") for b in range(B_LOC)]
            o1n = [bigp.tile([128, 4, N], f16, name=f"o1n{b}", tag=f"o1n{b}") for b in range(B_LOC)]
            o2t = [bigp.tile([128, NT, OUT], f16, name=f"o2t{b}", tag=f"o2t{b}") for b in range(B_LOC)]

            gts_o = outp.tile([128, B_LOC, 2, N], f16)
            node_o = outp.tile([128, B_LOC, NT, OUT], f16)
            out2_o = outp.tile([128, B_LOC, NT, OUT], f16)

            # ---- stage B: gts in [o, n] layout (bias per-partition) ----
            for b in range(B_LOC):
                for ot in range(2):
                    for nh in range(2):
                        ps = mmp.tile([128, MID], f32, tag="ps")
                        p5 = ps[:, :512]
                        for cc in range(2):
                            nc.tensor.matmul(
                                p5,
                                lhsT=wg_t[:, cc, ot * 128 : (ot + 1) * 128],
                                rhs=gtT_t[:, b, cc, nh * 512 : (nh + 1) * 512],
                                start=(cc == 0), stop=(cc == 1),
                            )
                        nc.scalar.activation(
                            out=gts_o[:, b, ot, nh * 512 : (nh + 1) * 512],
                            in_=p5, func=AF.Relu,
                            bias=bgcol_t[:, ot : ot + 1], scale=1.0,
                        )
            nc.gpsimd.dma_start(
                out=gtsT_d.rearrange("b (o p) n -> p b o n", p=128), in_=gts_o
            )

            # ---- stage C: gconv1 -> o1^T [j, m] ----
            for b in range(B_LOC):
                for jt in range(NT):
                    ps = mmp.tile([128, MID], f32, tag="ps")
                    nc.tensor.matmul(ps, lhsT=ones_t, rhs=b1_t, start=True, stop=False)
                    for cc in range(2):
                        nc.tensor.matmul(
                            ps,
                            lhsT=xT_t[:, b, cc, jt * 128 : (jt + 1) * 128],
                            rhs=w1_t[:, cc, :],
                            start=False, stop=(cc == 1),
                        )
                    nc.scalar.activation(out=o1t[b][:, jt, :], in_=ps, func=AF.Relu)

            # ---- stage A: atten^T = sigmoid * msT, per j-tile ----
            for b in range(B_LOC):
                for jt in range(NT):
                    sg = sgp.tile([128, N], f16, tag="sg")
                    nc.scalar.activation(
                        out=sg, in_=lirow_t[:, b, :], func=AF.Sigmoid,
                        bias=ljT_t[:, b, jt : jt + 1], scale=1.0,
                    )
                    nc.gpsimd.tensor_tensor(
                        out=At[b][jt], in0=At[b][jt], in1=sg, op=OP.mult
                    )

            # ---- stages D/E/F per batch ----
            for b in range(B_LOC):
                # D: o1m^T = atten^T-contraction + ln1 + residual -> o1nT
                for w in range(2):  # waves of 4 i-tiles
                    its = [w * 4 + k for k in range(4)]
                    pss = [mmp.tile([128, MID], f32, name="psw", tag="ps") for _ in its]
                    for jc in range(NT):
                        for k, it in enumerate(its):
                            nc.tensor.matmul(
                                pss[k],
                                lhsT=At[b][jc][:, it * 128 : (it + 1) * 128],
                                rhs=o1t[b][:, jc, :],
                                start=(jc == 0), stop=(jc == NT - 1),
                            )
                    mvw = workp.tile([128, 2, 4], f32, tag="mvw")
                    for k, it in enumerate(its):
                        sv = workp.tile([128, 6], f32, tag="sv")
                        nc.vector.bn_stats(out=sv, in_=pss[k])
                        nc.vector.bn_aggr(out=mvw[:, :, k], in_=sv)
                    stdw = workp.tile([128, 4], f32, tag="stdw")
                    nc.scalar.activation(
                        out=stdw, in_=mvw[:, 1, :], func=AF.Sqrt, bias=eps_t
                    )
                    rstdw = workp.tile([128, 4], f32, tag="rstdw")
                    nc.vector.reciprocal(out=rstdw, in_=stdw)
                    for k, it in enumerate(its):
                        outer = workp.tile([128, MID], f16, tag="outer")
                        nc.gpsimd.tensor_scalar_mul(
                            outer, g1row_t, rstdw[:, k : k + 1]
                        )
                        ln = workp.tile([128, MID], f16, tag="ln")
                        nc.vector.scalar_tensor_tensor(
                            out=ln, in0=pss[k], scalar=mvw[:, 0, k : k + 1],
                            in1=outer, op0=OP.subtract, op1=OP.mult,
                        )
                        if beta1_nz:
                            nc.gpsimd.tensor_tensor(
                                out=ln, in0=ln, in1=beta1_t, op=OP.add
                            )
                        nc.gpsimd.tensor_tensor(
                            out=o1nT[b][:, it, :], in0=ln, in1=o1t[b][:, it, :],
                            op=OP.add,
                        )

                # E: transpose o1_new -> [m, j], gconv2 -> o2^T
                for mc in range(4):
                    tp = tpp.tile([128, N], f16, tag="tp")
                    for it in range(NT):
                        nc.tensor.transpose(
                            tp[:, it * 128 : (it + 1) * 128],
                            o1nT[b][:, it, mc * 128 : (mc + 1) * 128],
                            ident_t,
                        )
                    nc.vector.tensor_copy(out=o1n[b][:, mc, :], in_=tp)
                for jt in range(NT):
                    ps = mmp.tile([128, MID], f32, tag="ps")
                    p256 = ps[:, :OUT]
                    nc.tensor.matmul(p256, lhsT=ones_t, rhs=b2_t, start=True, stop=False)
                    for mc in range(4):
                        nc.tensor.matmul(
                            p256,
                            lhsT=o1n[b][:, mc, jt * 128 : (jt + 1) * 128],
                            rhs=w2_t[:, mc, :],
                            start=False, stop=(mc == 3),
                        )
                    nc.vector.tensor_scalar_max(o2t[b][:, jt, :], p256, 0.0)

                # F: o2m^T contraction + ln2 -> node_feat, output2
                for w in range(2):
                    its = [w * 4 + k for k in range(4)]
                    pss = [mmp.tile([128, MID], f32, name="psw", tag="ps") for _ in its]
                    for jc in range(NT):
                        for k, it in enumerate(its):
                            nc.tensor.matmul(
                                pss[k][:, :OUT],
                                lhsT=At[b][jc][:, it * 128 : (it + 1) * 128],
                                rhs=o2t[b][:, jc, :],
                                start=(jc == 0), stop=(jc == NT - 1),
                            )
                    mvw = workp.tile([128, 2, 4], f32, tag="mvw")
                    for k, it in enumerate(its):
                        sv = workp.tile([128, 6], f32, tag="sv")
                        nc.vector.bn_stats(out=sv, in_=pss[k][:, :OUT])
                        nc.vector.bn_aggr(out=mvw[:, :, k], in_=sv)
                    stdw = workp.tile([128, 4], f32, tag="stdw")
                    nc.scalar.activation(
                        out=stdw, in_=mvw[:, 1, :], func=AF.Sqrt, bias=eps_t
                    )
                    rstdw = workp.tile([128, 4], f32, tag="rstdw")
                    nc.vector.reciprocal(out=rstdw, in_=stdw)
                    for k, it in enumerate(its):
                        outer = workp.tile([128, OUT], f16, tag="outer2")
                        nc.gpsimd.tensor_scalar_mul(
                            outer, g2row_t, rstdw[:, k : k + 1]
                        )
                        nf = node_o[:, b, it, :]
                        nc.vector.scalar_tensor_tensor(
                            out=nf, in0=pss[k][:, :OUT],
                            scalar=mvw[:, 0, k : k + 1],
                            in1=outer, op0=OP.subtract, op1=OP.mult,
                        )
                        if beta2_nz:
                            nc.gpsimd.tensor_tensor(
                                out=nf, in0=nf, in1=beta2_t, op=OP.add
                            )
                        nc.gpsimd.tensor_tensor(
                            out=out2_o[:, b, it, :], in0=nf, in1=o2t[b][:, it, :],
                            op=OP.add,
                        )

            nc.gpsimd.dma_start(
                out=node_d.rearrange("b (t p) o -> p b t o", p=128), in_=node_o
            )
            nc.gpsimd.dma_start(
                out=out2_d.rearrange("b (t p) o -> p b t o", p=128), in_=out2_o
            )

    nc.compile()
    return nc


def _compute_col_fast(m1, m2, sm):
    """Exact col == ones proof via a cheap sufficient condition, else None."""
    if m1.min() < 0.0 or m2.min() < 0.0 or sm.min() < 0.0:
        return None
    spos = (sm > 0).astype(F32)
    colnz = np.zeros(N, dtype=bool)
    nz1max = 0.0
    nz2max = 0.0
    for b in range(B):
        p1 = (m1[b] > 0).astype(F32)
        p2 = (m2[b] > 0).astype(F32)
        nz1max = max(nz1max, float((p1 @ spos[b]).max()))
        nz2max = max(nz2max, float((p2 @ spos[b]).max()))
        colnz |= ((p1 + p2).max(axis=0) > 0) & (spos[b] > 0)
    if nz1max <= CHILDS // 4 and nz2max <= CHILDS // 2 and colnz.all():
        return np.ones(N, dtype=F32)
    return None


def _compute_col_slow(m1, m2, sm, li, lj):
    """Exact replica of the reference top-k column-union (numpy)."""
    k4, k2 = CHILDS // 4, CHILDS // 2
    col = np.zeros(N, dtype=bool)
    for b in range(B):
        logits = li[b][:, None] + lj[b][None, :]
        a = 1.0 / (1.0 + np.exp(-logits.astype(F32)))
        mr1 = m1[b] * sm[b][None, :]
        mr2 = m2[b] * sm[b][None, :]
        a1 = a * mr1
        a2 = a * mr2
        # lax.top_k ties -> lowest index; stable argsort on (-a) reproduces it.
        col[np.argsort(-a1, axis=1, kind="stable")[:, :k4].ravel()] = True
        col[np.argsort(a1, axis=1, kind="stable")[:, :k4].ravel()] = True
        col[np.argsort(-a2, axis=1, kind="stable")[:, :k2].ravel()] = True
        col[np.argsort(a2, axis=1, kind="stable")[:, :k4].ravel()] = True
    return col.astype(F32)


def kernel(**inputs):
    x = np.ascontiguousarray(np.asarray(inputs["x"], dtype=F32))
    m1 = np.asarray(inputs["masks_roi1"], dtype=F32)
    m2 = np.asarray(inputs["masks_roi2"], dtype=F32)
    sm = np.asarray(inputs["score_mask"], dtype=F32)
    gt = np.asarray(inputs["gt_feat"], dtype=F32)
    W_att = np.asarray(inputs["W_att"], dtype=F32)
    b_att = np.asarray(inputs["b_att"], dtype=F32)
    W1 = np.asarray(inputs["W1"], dtype=F32)
    b1 = np.asarray(inputs["b1"], dtype=F32)
    W2 = np.asarray(inputs["W2"], dtype=F32)
    b2 = np.asarray(inputs["b2"], dtype=F32)
    g1 = np.asarray(inputs["g1"], dtype=F32)
    beta1 = np.asarray(inputs["beta1"], dtype=F32)
    g2 = np.asarray(inputs["g2"], dtype=F32)
    beta2 = np.asarray(inputs["beta2"], dtype=F32)
    Wg = np.asarray(inputs["Wg"], dtype=F32)
    bg = np.asarray(inputs["bg"], dtype=F32)

    assert x.shape == (B, N, CIN) and W_att.shape == (2 * CIN, 1)

    # ---- host prep: tiny vector math + layout/dtype staging ----
    lj = x.reshape(B * N, CIN) @ W_att[:CIN, 0]
    lj = lj.reshape(B, N)
    li = x.reshape(B * N, CIN) @ W_att[CIN:, 0]
    li = li.reshape(B, N) + b_att[0]

    col = _compute_col_fast(m1, m2, sm)
    if col is None:
        col = _compute_col_slow(m1, m2, sm, li, lj)

    # One fused mask tensor: msT[j, i] = (m1+m2)[i, j] * score[j] * col[j],
    # with the f_diag term folded exactly onto the diagonal as
    # f[j] / sigmoid(l_jj)   (then the device's sigmoid multiply restores f).
    msT = (m1 + m2).transpose(0, 2, 1) * (sm * col[None, :])[:, :, None]
    f = (sm == 0).astype(F32)
    ldiag = li + lj  # l_jj = li[j] + lj[j]
    sig_diag = 1.0 / (1.0 + np.exp(-ldiag))
    didx = np.arange(N)
    msT[:, didx, didx] += f / sig_diag
    msT = np.ascontiguousarray(msT).astype(F16)

    xT = np.ascontiguousarray(x.transpose(0, 2, 1)).astype(F16)
    gtT = np.ascontiguousarray(gt.transpose(0, 2, 1)).astype(F16)
    lirow = li.astype(F16)
    ljT = np.ascontiguousarray(lj.reshape(B, NT, 128).transpose(0, 2, 1)).astype(F32)

    # Weights: block-diagonal transposed layouts for the grouped convs.
    w1bd = np.zeros((CIN, MID), dtype=F32)
    for g in range(G):
        w1bd[64 * g : 64 * (g + 1), 128 * g : 128 * (g + 1)] = W1[
            128 * g : 128 * (g + 1), :
        ].T
    w2bd = np.zeros((MID, OUT), dtype=F32)
    for g in range(G):
        w2bd[128 * g : 128 * (g + 1), 64 * g : 64 * (g + 1)] = W2[
            64 * g : 64 * (g + 1), :
        ].T
    wgK = np.ascontiguousarray(Wg.reshape(2, 128, OUT)).astype(F16)
    w1K = np.ascontiguousarray(w1bd.reshape(2, 128, MID)).astype(F16)
    w2K = np.ascontiguousarray(w2bd.reshape(4, 128, OUT)).astype(F16)

    shared = {
        "wgK": wgK,
        "w1K": w1K,
        "w2K": w2K,
        "bgcol": np.ascontiguousarray(bg.reshape(2, 128)).astype(F32),
        "b1row": b1.reshape(1, MID).astype(F16),
        "b2row": b2.reshape(1, OUT).astype(F16),
        "g1row": g1.reshape(1, MID).astype(F32),
        "g2row": g2.reshape(1, OUT).astype(F32),
        "beta1row": beta1.reshape(1, MID).astype(F32),
        "beta2row": beta2.reshape(1, OUT).astype(F32),
        "ident": np.eye(128, dtype=F16),
        "onescol": np.ones((1, 128), dtype=F16),
    }
    in_maps = []
    for c in range(NCORES):
        s = slice(B_LOC * c, B_LOC * (c + 1))
        in_maps.append(
            {
                "msT": msT[s],
                "xT": xT[s],
                "gtT": gtT[s],
                "lirow": lirow[s],
                "ljT": ljT[s],
                **shared,
            }
        )

    beta_key = (bool(np.any(beta1)), bool(np.any(beta2)))
    if beta_key not in _PROGRAM_CACHE:
        _PROGRAM_CACHE[beta_key] = _build_program(*beta_key)
    nc = _PROGRAM_CACHE[beta_key]

    global _LAST_IN_MAPS
    _LAST_IN_MAPS = in_maps

    from concourse.bass_utils import run_bass_kernel_spmd

    res = run_bass_kernel_spmd(nc, in_maps, core_ids=list(range(NCORES)))
    results = res.results if hasattr(res, "results") else res

    gts = np.concatenate(
        [results[c]["gtsT"].transpose(0, 2, 1) for c in range(NCORES)], axis=0
    )
    node_feat = np.concatenate([results[c]["node"] for c in range(NCORES)], axis=0)
    output2 = np.concatenate([results[c]["out2"] for c in range(NCORES)], axis=0)
    return output2.astype(F32), gts.astype(F32), node_feat.astype(F32)


# revision 11
# speedup vs baseline: 1.6561x; 1.6561x over previous
"""Trainium2 Bass kernel for nn_Graph_module_net_0_loss_type_18631568130084.

GNN message-passing block:
  gts       = relu(gt_feat @ Wg + bg)
  attn[i,j] = sigmoid(x[j]@Wq + x[i]@Wk + b_att)          (H == 1)
  atten     = (attn * (mr1+mr2) * col + f_diag) / CHILDS  ([B,H,Nj,Ni])
  o1 = relu(gconv1(x^T)); o1 += ln1(o1 @ atten)^T
  o2 = relu(gconv2(o1));  node_feat = ln2(o2 @ atten);  output2 = (o2 + node_feat^T)^T

Sharding: data-parallel over batch B=16 -> 2 batches per core on 8 cores.

v3 design (v1 baseline 180us, v2 322us - gpsimd tensor ops are ~3-8x slower
than DVE, so v3 minimizes elementwise passes and keeps PSUM work on DVE/ACT):
 * ONE mask tensor: host pre-folds (m1+m2)*score*col into msT fp16 and folds
   the f_diag term exactly onto the diagonal as f[j]/sigmoid(l_jj); device
   atten^T = sigmoid_tile * msT_tile (one gpsimd TT per j-tile).
 * gts in [OUT, N] layout: bg is a per-partition bias fused into ACT relu.
 * gconv1 computed in BOTH layouts: o1t [j,m] (stage-D rhs) and o1mn [m,n]
   (per-partition bias b1 free in ACT relu).  LN1 apply is split:
   z = (o1m - mean)*rstd (one DVE tensor_scalar, 2 scalar operands), PE
   transposes z, and o1_new^T[m,j] = z^T * g1[m] + o1mn is ONE DVE
   scalar_tensor_tensor from PSUM - the gamma-multiply, residual add and
   PSUM->SBUF copy all fused into the transpose epilogue.
 * LN rstd: variances of 4 i-tiles packed into [128,4], one ACT Sqrt
   (bias=eps) + one DVE reciprocal per wave.  All sigmoids (both batches)
   run before any sqrt => exactly 2 ACT table loads.
 * DMA: few large transfers; f16 consts + x on sync ahead of masks,
   lirow/ljT on the ACT queue so sigmoids start ~2us in; outputs fp16 on
   gpsimd at the end (host casts back to fp32 / un-transposes gts).
 * Stage D matmuls issue jc-outer in waves of 4 i-tiles so PE starts
   contracting as soon as At[jc] is ready; 1/CHILDS cancels in both
   layernorms (eps rescaled by CHILDS^2).
 * The top-k "col" mask is computed exactly on the host: a cheap sufficient
   condition proves col == all-ones; otherwise an exact numpy replica runs.
"""

import numpy as np

B = 16
N = 1024
CIN = 256
MID = 512
OUT = 256
G = 4
CHILDS = 512
NCORES = 8
B_LOC = B // NCORES  # 2
NT = N // 128  # 8
EPS_LN = 1e-6 * float(CHILDS) ** 2  # eps rescaled because we drop the 1/CHILDS

F16 = np.float16
F32 = np.float32

_PROGRAM_CACHE = {}


def _build_program(beta1_nz: bool, beta2_nz: bool):
    import concourse.bacc as bacc
    import concourse.tile as tile
    from concourse import mybir

    f16 = mybir.dt.float16
    f32 = mybir.dt.float32
    AF = mybir.ActivationFunctionType
    OP = mybir.AluOpType

    nc = bacc.Bacc("TRN2", debug=False)

    def din(name, shape, dt):
        return nc.dram_tensor(name, shape, dt, kind="ExternalInput").ap()

    def dout(name, shape, dt):
        return nc.dram_tensor(name, shape, dt, kind="ExternalOutput").ap()

    # Per-core inputs (leading dim B_LOC where batch-dependent).
    msT_d = din("msT", [B_LOC, N, N], f16)       # (m1+m2)*score*col (+diag) ^T
    xT_d = din("xT", [B_LOC, CIN, N], f16)       # x^T   [c, n]
    gtT_d = din("gtT", [B_LOC, CIN, N], f16)     # gt^T  [c, n]
    lirow_d = din("lirow", [B_LOC, N], f16)      # x@Wk + b_att      (per-i row)
    ljT_d = din("ljT", [B_LOC, 128, NT], f32)    # x@Wq chunked      (per-j bias)
    # Replicated weights.
    wg_d = din("wgK", [2, 128, OUT], f16)        # Wg   (c-chunks)
    w1_d = din("w1K", [2, 128, MID], f16)        # block-diag W1^T (c-chunks)
    w2_d = din("w2K", [4, 128, OUT], f16)        # block-diag W2^T (m-chunks)
    bgcol_d = din("bgcol", [2, 128], f32)        # bg per o-tile (per-partition)
    b1col_d = din("b1col", [4, 128], f32)        # b1 per m-chunk (per-partition)
    g1col_d = din("g1col", [4, 128], f32)        # g1 per m-chunk (per-partition)
    b1_d = din("b1row", [1, MID], f16)
    b2_d = din("b2row", [1, OUT], f16)
    g2r16_d = din("g2row16", [1, OUT], f16)
    beta1col_d = din("beta1col", [4, 128], f32)
    beta2_d = din("beta2row", [1, OUT], f16)
    ident_d = din("ident", [128, 128], f16)
    ones_d = din("onescol", [1, 128], f16)

    gtsT_d = dout("gtsT", [B_LOC, OUT, N], f16)  # [o, n] - host un-transposes
    node_d = dout("node", [B_LOC, N, OUT], f16)
    out2_d = dout("out2", [B_LOC, N, OUT], f16)

    with tile.TileContext(nc) as tc:
        with tc.tile_pool(name="const", bufs=1) as constp, \
             tc.tile_pool(name="inp", bufs=1) as inp, \
             tc.tile_pool(name="at", bufs=1) as atp, \
             tc.tile_pool(name="big", bufs=1) as bigp, \
             tc.tile_pool(name="work", bufs=4) as workp, \
             tc.tile_pool(name="sg", bufs=3) as sgp, \
             tc.tile_pool(name="outs", bufs=1) as outp, \
             tc.tile_pool(name="mm", bufs=6, space="PSUM") as mmp, \
             tc.tile_pool(name="tp", bufs=2, space="PSUM") as tpp:

            # ---- f16 weights + x early on sync (tiny transfers, needed by
            # B/C within ~10us) ----
            ident_t = constp.tile([128, 128], f16)
            nc.sync.dma_start(out=ident_t, in_=ident_d)
            ones_t = constp.tile([1, 128], f16)
            nc.sync.dma_start(out=ones_t, in_=ones_d)
            wg_t = constp.tile([128, 2, OUT], f16)
            nc.sync.dma_start(out=wg_t, in_=wg_d.rearrange("c p f -> p c f"))
            w1_t = constp.tile([128, 2, MID], f16)
            nc.sync.dma_start(out=w1_t, in_=w1_d.rearrange("c p f -> p c f"))
            w2_t = constp.tile([128, 4, OUT], f16)
            nc.sync.dma_start(out=w2_t, in_=w2_d.rearrange("c p f -> p c f"))
            b1_t = constp.tile([1, MID], f16)
            nc.sync.dma_start(out=b1_t, in_=b1_d)
            b2_t = constp.tile([1, OUT], f16)
            nc.sync.dma_start(out=b2_t, in_=b2_d)
            xT_t = inp.tile([128, B_LOC, 2, N], f16)
            nc.sync.dma_start(
                out=xT_t, in_=xT_d.rearrange("b (c p) n -> p b c n", p=128)
            )

            # masks batch-0 first, then gt (needed ~15us in), then batch-1
            At = [
                [atp.tile([128, N], f16, name=f"At{b}_{jt}", tag=f"At{b}_{jt}")
                 for jt in range(NT)]
                for b in range(B_LOC)
            ]
            for jt in range(NT):
                nc.sync.dma_start(
                    out=At[0][jt], in_=msT_d[0, jt * 128 : (jt + 1) * 128, :]
                )
            gtT_t = inp.tile([128, B_LOC, 2, N], f16)
            nc.sync.dma_start(
                out=gtT_t, in_=gtT_d.rearrange("b (c p) n -> p b c n", p=128)
            )
            for jt in range(NT):
                nc.sync.dma_start(
                    out=At[1][jt], in_=msT_d[1, jt * 128 : (jt + 1) * 128, :]
                )

            # ---- ACT queue: lirow/ljT first so sigmoids start immediately ----
            lirow_t = inp.tile([128, B_LOC, N], f16)
            nc.scalar.dma_start(
                out=lirow_t, in_=lirow_d[None].to_broadcast([128, B_LOC, N])
            )
            ljT_t = inp.tile([128, B_LOC, NT], f32)
            nc.scalar.dma_start(out=ljT_t, in_=ljT_d.rearrange("b p t -> p b t"))

            # ---- f32 / late-needed consts on gpsimd (idle until ~12us) ----
            bgcol_t = constp.tile([128, 2], f32)
            nc.gpsimd.dma_start(out=bgcol_t, in_=bgcol_d.rearrange("o p -> p o"))
            b1col_t = constp.tile([128, 4], f32)
            nc.gpsimd.dma_start(out=b1col_t, in_=b1col_d.rearrange("c p -> p c"))
            g1col_t = constp.tile([128, 4], f32)
            nc.gpsimd.dma_start(out=g1col_t, in_=g1col_d.rearrange("c p -> p c"))
            g2r16_t = constp.tile([128, OUT], f16)
            nc.gpsimd.dma_start(out=g2r16_t, in_=g2r16_d.to_broadcast([128, OUT]))
            if beta1_nz:
                beta1col_t = constp.tile([128, 4], f32)
                nc.gpsimd.dma_start(
                    out=beta1col_t, in_=beta1col_d.rearrange("c p -> p c")
                )
            if beta2_nz:
                beta2_t = constp.tile([128, OUT], f16)
                nc.gpsimd.dma_start(out=beta2_t, in_=beta2_d.to_broadcast([128, OUT]))
            eps_t = constp.tile([128, 1], f32)
            nc.vector.memset(eps_t, EPS_LN)

            # Per-batch activation tensors (both batches resident).
            o1t = [bigp.tile([128, NT, MID], f16, name=f"o1t{b}", tag=f"o1t{b}")
                   for b in range(B_LOC)]
            o1mn = [bigp.tile([128, 4, N], f16, name=f"o1mn{b}", tag=f"o1mn{b}")
                    for b in range(B_LOC)]
            zt = [bigp.tile([128, NT, MID], f16, name=f"zt{b}", tag=f"zt{b}")
                  for b in range(B_LOC)]
            o1n = [bigp.tile([128, 4, N], f16, name=f"o1n{b}", tag=f"o1n{b}")
                   for b in range(B_LOC)]
            o2t = [bigp.tile([128, NT, OUT], f16, name=f"o2t{b}", tag=f"o2t{b}")
                   for b in range(B_LOC)]

            gts_o = outp.tile([128, B_LOC, 2, N], f16)
            node_o = outp.tile([128, B_LOC, NT, OUT], f16)
            out2_o = outp.tile([128, B_LOC, NT, OUT], f16)

            # ---- stage A (both batches): atten^T = sigmoid * msT ----
            for b in range(B_LOC):
                for jt in range(NT):
                    sg = sgp.tile([128, N], f16, tag="sg")
                    nc.scalar.activation(
                        out=sg, in_=lirow_t[:, b, :], func=AF.Sigmoid,
                        bias=ljT_t[:, b, jt : jt + 1], scale=1.0,
                    )
                    nc.gpsimd.tensor_tensor(
                        out=At[b][jt], in0=At[b][jt], in1=sg, op=OP.mult
                    )

            for b in range(B_LOC):
                # ---- stage C: gconv1 -> o1t [j, m] ----
                for jt in range(NT):
                    ps = mmp.tile([128, MID], f32, tag="ps")
                    nc.tensor.matmul(ps, lhsT=ones_t, rhs=b1_t, start=True, stop=False)
                    for cc in range(2):
                        nc.tensor.matmul(
                            ps,
                            lhsT=xT_t[:, b, cc, jt * 128 : (jt + 1) * 128],
                            rhs=w1_t[:, cc, :],
                            start=False, stop=(cc == 1),
                        )
                    nc.scalar.activation(out=o1t[b][:, jt, :], in_=ps, func=AF.Relu)
                # ---- stage CT: gconv1 -> o1mn [m, n] (bias per-partition) ----
                for mc in range(4):
                    for nh in range(2):
                        ps = mmp.tile([128, MID], f32, tag="ps")
                        for cc in range(2):
                            nc.tensor.matmul(
                                ps,
                                lhsT=w1_t[:, cc, mc * 128 : (mc + 1) * 128],
                                rhs=xT_t[:, b, cc, nh * 512 : (nh + 1) * 512],
                                start=(cc == 0), stop=(cc == 1),
                            )
                        nc.scalar.activation(
                            out=o1mn[b][:, mc, nh * 512 : (nh + 1) * 512],
                            in_=ps, func=AF.Relu,
                            bias=b1col_t[:, mc : mc + 1], scale=1.0,
                        )
                # ---- stage B: gts in [o, n] layout ----
                for ot in range(2):
                    for nh in range(2):
                        ps = mmp.tile([128, MID], f32, tag="ps")
                        p5 = ps[:, :512]
                        for cc in range(2):
                            nc.tensor.matmul(
                                p5,
                                lhsT=wg_t[:, cc, ot * 128 : (ot + 1) * 128],
                                rhs=gtT_t[:, b, cc, nh * 512 : (nh + 1) * 512],
                                start=(cc == 0), stop=(cc == 1),
                            )
                        nc.scalar.activation(
                            out=gts_o[:, b, ot, nh * 512 : (nh + 1) * 512],
                            in_=p5, func=AF.Relu,
                            bias=bgcol_t[:, ot : ot + 1], scale=1.0,
                        )

            for b in range(B_LOC):
                # ---- stage D: o1m^T contraction + z = (o1m - mean)*rstd ----
                for w in range(2):  # waves of 4 i-tiles
                    its = [w * 4 + k for k in range(4)]
                    pss = [mmp.tile([128, MID], f32, name="psw", tag="ps")
                           for _ in its]
                    for jc in range(NT):
                        for k, it in enumerate(its):
                            nc.tensor.matmul(
                                pss[k],
                                lhsT=At[b][jc][:, it * 128 : (it + 1) * 128],
                                rhs=o1t[b][:, jc, :],
                                start=(jc == 0), stop=(jc == NT - 1),
                            )
                    mvw = workp.tile([128, 2, 4], f32, tag="mvw")
                    for k, it in enumerate(its):
                        sv = workp.tile([128, 6], f32, tag="sv")
                        nc.vector.bn_stats(out=sv, in_=pss[k])
                        nc.vector.bn_aggr(out=mvw[:, :, k], in_=sv)
                    stdw = workp.tile([128, 4], f32, tag="stdw")
                    nc.scalar.activation(
                        out=stdw, in_=mvw[:, 1, :], func=AF.Sqrt, bias=eps_t
                    )
                    rstdw = workp.tile([128, 4], f32, tag="rstdw")
                    nc.vector.reciprocal(out=rstdw, in_=stdw)
                    for k, it in enumerate(its):
                        nc.vector.tensor_scalar(
                            out=zt[b][:, it, :], in0=pss[k],
                            scalar1=mvw[:, 0, k : k + 1],
                            scalar2=rstdw[:, k : k + 1],
                            op0=OP.subtract, op1=OP.mult,
                        )

                # ---- stage E: transpose z, fused *g1 + o1mn -> o1n [m, j];
                #      then gconv2 -> o2t [j, o] ----
                for mc in range(4):
                    tp = tpp.tile([128, N], f16, tag="tp")
                    for it in range(NT):
                        nc.tensor.transpose(
                            tp[:, it * 128 : (it + 1) * 128],
                            zt[b][:, it, mc * 128 : (mc + 1) * 128],
                            ident_t,
                        )
                    nc.vector.scalar_tensor_tensor(
                        out=o1n[b][:, mc, :], in0=tp,
                        scalar=g1col_t[:, mc : mc + 1],
                        in1=o1mn[b][:, mc, :], op0=OP.mult, op1=OP.add,
                    )
                    if beta1_nz:
                        nc.gpsimd.tensor_scalar_add(
                            o1n[b][:, mc, :], o1n[b][:, mc, :],
                            beta1col_t[:, mc : mc + 1],
                        )
                for jt in range(NT):
                    ps = mmp.tile([128, MID], f32, tag="ps")
                    p256 = ps[:, :OUT]
                    nc.tensor.matmul(p256, lhsT=ones_t, rhs=b2_t, start=True, stop=False)
                    for mc in range(4):
                        nc.tensor.matmul(
                            p256,
                            lhsT=o1n[b][:, mc, jt * 128 : (jt + 1) * 128],
                            rhs=w2_t[:, mc, :],
                            start=False, stop=(mc == 3),
                        )
                    nc.vector.tensor_scalar_max(o2t[b][:, jt, :], p256, 0.0)

                # ---- stage F: o2m^T contraction + ln2 -> node_feat, output2 ----
                for w in range(2):
                    its = [w * 4 + k for k in range(4)]
                    pss = [mmp.tile([128, MID], f32, name="psw", tag="ps")
                           for _ in its]
                    for jc in range(NT):
                        for k, it in enumerate(its):
                            nc.tensor.matmul(
                                pss[k][:, :OUT],
                                lhsT=At[b][jc][:, it * 128 : (it + 1) * 128],
                                rhs=o2t[b][:, jc, :],
                                start=(jc == 0), stop=(jc == NT - 1),
                            )
                    mvw = workp.tile([128, 2, 4], f32, tag="mvw")
                    for k, it in enumerate(its):
                        sv = workp.tile([128, 6], f32, tag="sv")
                        nc.vector.bn_stats(out=sv, in_=pss[k][:, :OUT])
                        nc.vector.bn_aggr(out=mvw[:, :, k], in_=sv)
                    stdw = workp.tile([128, 4], f32, tag="stdw")
                    nc.scalar.activation(
                        out=stdw, in_=mvw[:, 1, :], func=AF.Sqrt, bias=eps_t
                    )
                    rstdw = workp.tile([128, 4], f32, tag="rstdw")
                    nc.vector.reciprocal(out=rstdw, in_=stdw)
                    for k, it in enumerate(its):
                        z2 = workp.tile([128, OUT], f16, tag="z2")
                        nc.vector.tensor_scalar(
                            out=z2, in0=pss[k][:, :OUT],
                            scalar1=mvw[:, 0, k : k + 1],
                            scalar2=rstdw[:, k : k + 1],
                            op0=OP.subtract, op1=OP.mult,
                        )
                        nf = node_o[:, b, it, :]
                        nc.gpsimd.tensor_tensor(
                            out=nf, in0=z2, in1=g2r16_t, op=OP.mult
                        )
                        if beta2_nz:
                            nc.gpsimd.tensor_tensor(
                                out=nf, in0=nf, in1=beta2_t, op=OP.add
                            )
                        nc.gpsimd.tensor_tensor(
                            out=out2_o[:, b, it, :], in0=nf,
                            in1=o2t[b][:, it, :], op=OP.add,
                        )

            nc.gpsimd.dma_start(
                out=gtsT_d.rearrange("b (o p) n -> p b o n", p=128), in_=gts_o
            )
            nc.gpsimd.dma_start(
                out=node_d.rearrange("b (t p) o -> p b t o", p=128), in_=node_o
            )
            nc.gpsimd.dma_start(
                out=out2_d.rearrange("b (t p) o -> p b t o", p=128), in_=out2_o
            )

    nc.compile()
    return nc


def _compute_col_fast(m1, m2, sm):
    """Exact col == ones proof via a cheap sufficient condition, else None."""
    if m1.min() < 0.0 or m2.min() < 0.0 or sm.min() < 0.0:
        return None
    spos = (sm > 0).astype(F32)
    colnz = np.zeros(N, dtype=bool)
    nz1max = 0.0
    nz2max = 0.0
    for b in range(B):
        p1 = (m1[b] > 0).astype(F32)
        p2 = (m2[b] > 0).astype(F32)
        nz1max = max(nz1max, float((p1 @ spos[b]).max()))
        nz2max = max(nz2max, float((p2 @ spos[b]).max()))
        colnz |= ((p1 + p2).max(axis=0) > 0) & (spos[b] > 0)
    if nz1max <= CHILDS // 4 and nz2max <= CHILDS // 2 and colnz.all():
        return np.ones(N, dtype=F32)
    return None


def _compute_col_slow(m1, m2, sm, li, lj):
    """Exact replica of the reference top-k column-union (numpy)."""
    k4, k2 = CHILDS // 4, CHILDS // 2
    col = np.zeros(N, dtype=bool)
    for b in range(B):
        logits = li[b][:, None] + lj[b][None, :]
        a = 1.0 / (1.0 + np.exp(-logits.astype(F32)))
        mr1 = m1[b] * sm[b][None, :]
        mr2 = m2[b] * sm[b][None, :]
        a1 = a * mr1
        a2 = a * mr2
        # lax.top_k ties -> lowest index; stable argsort on (-a) reproduces it.
        col[np.argsort(-a1, axis=1, kind="stable")[:, :k4].ravel()] = True
        col[np.argsort(a1, axis=1, kind="stable")[:, :k4].ravel()] = True
        col[np.argsort(-a2, axis=1, kind="stable")[:, :k2].ravel()] = True
        col[np.argsort(a2, axis=1, kind="stable")[:, :k4].ravel()] = True
    return col.astype(F32)


def kernel(**inputs):
    x = np.ascontiguousarray(np.asarray(inputs["x"], dtype=F32))
    m1 = np.asarray(inputs["masks_roi1"], dtype=F32)
    m2 = np.asarray(inputs["masks_roi2"], dtype=F32)
    sm = np.asarray(inputs["score_mask"], dtype=F32)
    gt = np.asarray(inputs["gt_feat"], dtype=F32)
    W_att = np.asarray(inputs["W_att"], dtype=F32)
    b_att = np.asarray(inputs["b_att"], dtype=F32)
    W1 = np.asarray(inputs["W1"], dtype=F32)
    b1 = np.asarray(inputs["b1"], dtype=F32)
    W2 = np.asarray(inputs["W2"], dtype=F32)
    b2 = np.asarray(inputs["b2"], dtype=F32)
    g1 = np.asarray(inputs["g1"], dtype=F32)
    beta1 = np.asarray(inputs["beta1"], dtype=F32)
    g2 = np.asarray(inputs["g2"], dtype=F32)
    beta2 = np.asarray(inputs["beta2"], dtype=F32)
    Wg = np.asarray(inputs["Wg"], dtype=F32)
    bg = np.asarray(inputs["bg"], dtype=F32)

    assert x.shape == (B, N, CIN) and W_att.shape == (2 * CIN, 1)

    # ---- host prep: tiny vector math + layout/dtype staging ----
    lj = x.reshape(B * N, CIN) @ W_att[:CIN, 0]
    lj = lj.reshape(B, N)
    li = x.reshape(B * N, CIN) @ W_att[CIN:, 0]
    li = li.reshape(B, N) + b_att[0]

    col = _compute_col_fast(m1, m2, sm)
    if col is None:
        col = _compute_col_slow(m1, m2, sm, li, lj)

    # One fused mask tensor: msT[j, i] = (m1+m2)[i, j] * score[j] * col[j],
    # with the f_diag term folded exactly onto the diagonal as
    # f[j] / sigmoid(l_jj)   (then the device's sigmoid multiply restores f).
    msT = (m1 + m2).transpose(0, 2, 1) * (sm * col[None, :])[:, :, None]
    f = (sm == 0).astype(F32)
    ldiag = li + lj  # l_jj = li[j] + lj[j]
    sig_diag = 1.0 / (1.0 + np.exp(-ldiag))
    didx = np.arange(N)
    msT[:, didx, didx] += f / sig_diag
    msT = np.ascontiguousarray(msT).astype(F16)

    xT = np.ascontiguousarray(x.transpose(0, 2, 1)).astype(F16)
    gtT = np.ascontiguousarray(gt.transpose(0, 2, 1)).astype(F16)
    lirow = li.astype(F16)
    ljT = np.ascontiguousarray(lj.reshape(B, NT, 128).transpose(0, 2, 1)).astype(F32)

    # Weights: block-diagonal transposed layouts for the grouped convs.
    w1bd = np.zeros((CIN, MID), dtype=F32)
    for g in range(G):
        w1bd[64 * g : 64 * (g + 1), 128 * g : 128 * (g + 1)] = W1[
            128 * g : 128 * (g + 1), :
        ].T
    w2bd = np.zeros((MID, OUT), dtype=F32)
    for g in range(G):
        w2bd[128 * g : 128 * (g + 1), 64 * g : 64 * (g + 1)] = W2[
            64 * g : 64 * (g + 1), :
        ].T
    wgK = np.ascontiguousarray(Wg.reshape(2, 128, OUT)).astype(F16)
    w1K = np.ascontiguousarray(w1bd.reshape(2, 128, MID)).astype(F16)
    w2K = np.ascontiguousarray(w2bd.reshape(4, 128, OUT)).astype(F16)

    shared = {
        "wgK": wgK,
        "w1K": w1K,
        "w2K": w2K,
        "bgcol": np.ascontiguousarray(bg.reshape(2, 128)).astype(F32),
        "b1col": np.ascontiguousarray(b1.reshape(4, 128)).astype(F32),
        "g1col": np.ascontiguousarray(g1.reshape(4, 128)).astype(F32),
        "b1row": b1.reshape(1, MID).astype(F16),
        "b2row": b2.reshape(1, OUT).astype(F16),
        "g2row16": g2.reshape(1, OUT).astype(F16),
        "beta1col": np.ascontiguousarray(beta1.reshape(4, 128)).astype(F32),
        "beta2row": beta2.reshape(1, OUT).astype(F16),
        "ident": np.eye(128, dtype=F16),
        "onescol": np.ones((1, 128), dtype=F16),
    }
    in_maps = []
    for c in range(NCORES):
        s = slice(B_LOC * c, B_LOC * (c + 1))
        in_maps.append(
            {
                "msT": msT[s],
                "xT": xT[s],
                "gtT": gtT[s],
                "lirow": lirow[s],
                "ljT": ljT[s],
                **shared,
            }
        )

    beta_key = (bool(np.any(beta1)), bool(np.any(beta2)))
    if beta_key not in _PROGRAM_CACHE:
        _PROGRAM_CACHE[beta_key] = _build_program(*beta_key)
    nc = _PROGRAM_CACHE[beta_key]

    global _LAST_IN_MAPS
    _LAST_IN_MAPS = in_maps

    from concourse.bass_utils import run_bass_kernel_spmd

    res = run_bass_kernel_spmd(nc, in_maps, core_ids=list(range(NCORES)))
    results = res.results if hasattr(res, "results") else res

    gts = np.concatenate(
        [results[c]["gtsT"].transpose(0, 2, 1) for c in range(NCORES)], axis=0
    )
    node_feat = np.concatenate([results[c]["node"] for c in range(NCORES)], axis=0)
    output2 = np.concatenate([results[c]["out2"] for c in range(NCORES)], axis=0)
    return output2.astype(F32), gts.astype(F32), node_feat.astype(F32)


# revision 12
# speedup vs baseline: 1.7689x; 1.0681x over previous
"""Trainium2 Bass kernel for nn_Graph_module_net_0_loss_type_18631568130084.

GNN message-passing block:
  gts       = relu(gt_feat @ Wg + bg)
  attn[i,j] = sigmoid(x[j]@Wq + x[i]@Wk + b_att)          (H == 1)
  atten     = (attn * (mr1+mr2) * col + f_diag) / CHILDS  ([B,H,Nj,Ni])
  o1 = relu(gconv1(x^T)); o1 += ln1(o1 @ atten)^T
  o2 = relu(gconv2(o1));  node_feat = ln2(o2 @ atten);  output2 = (o2 + node_feat^T)^T

Sharding: data-parallel over batch B=16 -> 2 batches per core on 8 cores.

v3 design (v1 baseline 180us, v2 322us - gpsimd tensor ops are ~3-8x slower
than DVE, so v3 minimizes elementwise passes and keeps PSUM work on DVE/ACT):
 * ONE mask tensor: host pre-folds (m1+m2)*score*col into msT fp16 and folds
   the f_diag term exactly onto the diagonal as f[j]/sigmoid(l_jj); device
   atten^T = sigmoid_tile * msT_tile (one gpsimd TT per j-tile).
 * gts in [OUT, N] layout: bg is a per-partition bias fused into ACT relu.
 * gconv1 computed in BOTH layouts: o1t [j,m] (stage-D rhs) and o1mn [m,n]
   (per-partition bias b1 free in ACT relu).  LN1 apply is split:
   z = (o1m - mean)*rstd (one DVE tensor_scalar, 2 scalar operands), PE
   transposes z, and o1_new^T[m,j] = z^T * g1[m] + o1mn is ONE DVE
   scalar_tensor_tensor from PSUM - the gamma-multiply, residual add and
   PSUM->SBUF copy all fused into the transpose epilogue.
 * LN rstd: variances of 4 i-tiles packed into [128,4], one ACT Sqrt
   (bias=eps) + one DVE reciprocal per wave.  All sigmoids (both batches)
   run before any sqrt => exactly 2 ACT table loads.
 * DMA: few large transfers; f16 consts + x on sync ahead of masks,
   lirow/ljT on the ACT queue so sigmoids start ~2us in; outputs fp16 on
   gpsimd at the end (host casts back to fp32 / un-transposes gts).
 * Stage D matmuls issue jc-outer in waves of 4 i-tiles so PE starts
   contracting as soon as At[jc] is ready; 1/CHILDS cancels in both
   layernorms (eps rescaled by CHILDS^2).
 * The top-k "col" mask is computed exactly on the host: a cheap sufficient
   condition proves col == all-ones; otherwise an exact numpy replica runs.
"""

import numpy as np

B = 16
N = 1024
CIN = 256
MID = 512
OUT = 256
G = 4
CHILDS = 512
NCORES = 8
B_LOC = B // NCORES  # 2
NT = N // 128  # 8
EPS_LN = 1e-6 * float(CHILDS) ** 2  # eps rescaled because we drop the 1/CHILDS

F16 = np.float16
F32 = np.float32

_PROGRAM_CACHE = {}


def _build_program(beta1_nz: bool, beta2_nz: bool):
    import concourse.bacc as bacc
    import concourse.tile as tile
    from concourse import mybir

    f16 = mybir.dt.float16
    f32 = mybir.dt.float32
    AF = mybir.ActivationFunctionType
    OP = mybir.AluOpType

    nc = bacc.Bacc("TRN2", debug=False)

    def din(name, shape, dt):
        return nc.dram_tensor(name, shape, dt, kind="ExternalInput").ap()

    def dout(name, shape, dt):
        return nc.dram_tensor(name, shape, dt, kind="ExternalOutput").ap()

    # Per-core inputs (leading dim B_LOC where batch-dependent).
    msT_d = din("msT", [B_LOC, N, N], f16)       # (m1+m2)*score*col (+diag) ^T
    xT_d = din("xT", [B_LOC, CIN, N], f16)       # x^T   [c, n]
    gtT_d = din("gtT", [B_LOC, CIN, N], f16)     # gt^T  [c, n]
    lirow_d = din("lirow", [B_LOC, N], f16)      # x@Wk + b_att      (per-i row)
    ljT_d = din("ljT", [B_LOC, 128, NT], f32)    # x@Wq chunked      (per-j bias)
    # Replicated weights.
    wg_d = din("wgK", [2, 128, OUT], f16)        # Wg   (c-chunks)
    w1_d = din("w1K", [2, 128, MID], f16)        # block-diag W1^T (c-chunks)
    w2_d = din("w2K", [4, 128, OUT], f16)        # block-diag W2^T (m-chunks)
    bgcol_d = din("bgcol", [2, 128], f32)        # bg per o-tile (per-partition)
    b1col_d = din("b1col", [4, 128], f32)        # b1 per m-chunk (per-partition)
    g1col_d = din("g1col", [4, 128], f32)        # g1 per m-chunk (per-partition)
    b1_d = din("b1row", [1, MID], f16)
    b2_d = din("b2row", [1, OUT], f16)
    g2r16_d = din("g2row16", [1, OUT], f16)
    beta1col_d = din("beta1col", [4, 128], f32)
    beta2_d = din("beta2row", [1, OUT], f16)
    ident_d = din("ident", [128, 128], f16)
    ones_d = din("onescol", [1, 128], f16)

    gtsT_d = dout("gtsT", [B_LOC, OUT, N], f16)  # [o, n] - host un-transposes
    node_d = dout("node", [B_LOC, N, OUT], f16)
    out2_d = dout("out2", [B_LOC, N, OUT], f16)

    with tile.TileContext(nc) as tc:
        with tc.tile_pool(name="const", bufs=1) as constp, \
             tc.tile_pool(name="inp", bufs=1) as inp, \
             tc.tile_pool(name="at", bufs=1) as atp, \
             tc.tile_pool(name="big", bufs=1) as bigp, \
             tc.tile_pool(name="work", bufs=4) as workp, \
             tc.tile_pool(name="sg", bufs=3) as sgp, \
             tc.tile_pool(name="outs", bufs=1) as outp, \
             tc.tile_pool(name="mm", bufs=6, space="PSUM") as mmp, \
             tc.tile_pool(name="tp", bufs=2, space="PSUM") as tpp:

            # ---- f16 weights + x early on sync (tiny transfers, needed by
            # B/C within ~10us) ----
            ident_t = constp.tile([128, 128], f16)
            nc.sync.dma_start(out=ident_t, in_=ident_d)
            ones_t = constp.tile([1, 128], f16)
            nc.sync.dma_start(out=ones_t, in_=ones_d)
            wg_t = constp.tile([128, 2, OUT], f16)
            nc.sync.dma_start(out=wg_t, in_=wg_d.rearrange("c p f -> p c f"))
            w1_t = constp.tile([128, 2, MID], f16)
            nc.sync.dma_start(out=w1_t, in_=w1_d.rearrange("c p f -> p c f"))
            w2_t = constp.tile([128, 4, OUT], f16)
            nc.sync.dma_start(out=w2_t, in_=w2_d.rearrange("c p f -> p c f"))
            b1_t = constp.tile([1, MID], f16)
            nc.sync.dma_start(out=b1_t, in_=b1_d)
            b2_t = constp.tile([1, OUT], f16)
            nc.sync.dma_start(out=b2_t, in_=b2_d)
            xT_t = inp.tile([128, B_LOC, 2, N], f16)
            nc.sync.dma_start(
                out=xT_t, in_=xT_d.rearrange("b (c p) n -> p b c n", p=128)
            )

            # masks batch-0 first, then gt (needed ~15us in), then batch-1
            At = [
                [atp.tile([128, N], f16, name=f"At{b}_{jt}", tag=f"At{b}_{jt}")
                 for jt in range(NT)]
                for b in range(B_LOC)
            ]
            for jt in range(NT):
                nc.sync.dma_start(
                    out=At[0][jt], in_=msT_d[0, jt * 128 : (jt + 1) * 128, :]
                )
            gtT_t = inp.tile([128, B_LOC, 2, N], f16)
            nc.sync.dma_start(
                out=gtT_t, in_=gtT_d.rearrange("b (c p) n -> p b c n", p=128)
            )
            for jt in range(NT):
                nc.sync.dma_start(
                    out=At[1][jt], in_=msT_d[1, jt * 128 : (jt + 1) * 128, :]
                )

            # ---- ACT queue: lirow/ljT first so sigmoids start immediately ----
            lirow_t = inp.tile([128, B_LOC, N], f16)
            nc.scalar.dma_start(
                out=lirow_t, in_=lirow_d[None].to_broadcast([128, B_LOC, N])
            )
            ljT_t = inp.tile([128, B_LOC, NT], f32)
            nc.scalar.dma_start(out=ljT_t, in_=ljT_d.rearrange("b p t -> p b t"))

            # ---- f32 / late-needed consts on gpsimd (idle until ~12us) ----
            bgcol_t = constp.tile([128, 2], f32)
            nc.gpsimd.dma_start(out=bgcol_t, in_=bgcol_d.rearrange("o p -> p o"))
            b1col_t = constp.tile([128, 4], f32)
            nc.gpsimd.dma_start(out=b1col_t, in_=b1col_d.rearrange("c p -> p c"))
            g1col_t = constp.tile([128, 4], f32)
            nc.gpsimd.dma_start(out=g1col_t, in_=g1col_d.rearrange("c p -> p c"))
            g2r16_t = constp.tile([128, OUT], f16)
            nc.gpsimd.dma_start(out=g2r16_t, in_=g2r16_d.to_broadcast([128, OUT]))
            if beta1_nz:
                beta1col_t = constp.tile([128, 4], f32)
                nc.gpsimd.dma_start(
                    out=beta1col_t, in_=beta1col_d.rearrange("c p -> p c")
                )
            if beta2_nz:
                beta2_t = constp.tile([128, OUT], f16)
                nc.gpsimd.dma_start(out=beta2_t, in_=beta2_d.to_broadcast([128, OUT]))
            eps_t = constp.tile([128, 1], f32)
            nc.vector.memset(eps_t, EPS_LN)

            # Per-batch activation tensors (both batches resident).
            o1t = [bigp.tile([128, NT, MID], f16, name=f"o1t{b}", tag=f"o1t{b}")
                   for b in range(B_LOC)]
            o1mn = [bigp.tile([128, 4, N], f16, name=f"o1mn{b}", tag=f"o1mn{b}")
                    for b in range(B_LOC)]
            zt = [bigp.tile([128, NT, MID], f16, name=f"zt{b}", tag=f"zt{b}")
                  for b in range(B_LOC)]
            o1n = [bigp.tile([128, 4, N], f16, name=f"o1n{b}", tag=f"o1n{b}")
                   for b in range(B_LOC)]
            o2t = [bigp.tile([128, NT, OUT], f16, name=f"o2t{b}", tag=f"o2t{b}")
                   for b in range(B_LOC)]

            gts_o = outp.tile([128, B_LOC, 2, N], f16)
            node_o = outp.tile([128, B_LOC, NT, OUT], f16)
            out2_o = outp.tile([128, B_LOC, NT, OUT], f16)

            for b in range(B_LOC):
                # ---- stage A: atten^T = sigmoid * msT ----
                for jt in range(NT):
                    sg = sgp.tile([128, N], f16, tag="sg")
                    nc.scalar.activation(
                        out=sg, in_=lirow_t[:, b, :], func=AF.Sigmoid,
                        bias=ljT_t[:, b, jt : jt + 1], scale=1.0,
                    )
                    nc.vector.tensor_tensor(
                        out=At[b][jt], in0=At[b][jt], in1=sg, op=OP.mult
                    )
                # ---- stage C: gconv1 -> o1t [j, m] ----
                for jt in range(NT):
                    ps = mmp.tile([128, MID], f32, tag="ps")
                    nc.tensor.matmul(ps, lhsT=ones_t, rhs=b1_t, start=True, stop=False)
                    for cc in range(2):
                        nc.tensor.matmul(
                            ps,
                            lhsT=xT_t[:, b, cc, jt * 128 : (jt + 1) * 128],
                            rhs=w1_t[:, cc, :],
                            start=False, stop=(cc == 1),
                        )
                    nc.scalar.activation(out=o1t[b][:, jt, :], in_=ps, func=AF.Relu)
                # ---- stage CT: gconv1 -> o1mn [m, n] (bias per-partition) ----
                for mc in range(4):
                    for nh in range(2):
                        ps = mmp.tile([128, MID], f32, tag="ps")
                        for cc in range(2):
                            nc.tensor.matmul(
                                ps,
                                lhsT=w1_t[:, cc, mc * 128 : (mc + 1) * 128],
                                rhs=xT_t[:, b, cc, nh * 512 : (nh + 1) * 512],
                                start=(cc == 0), stop=(cc == 1),
                            )
                        nc.scalar.activation(
                            out=o1mn[b][:, mc, nh * 512 : (nh + 1) * 512],
                            in_=ps, func=AF.Relu,
                            bias=b1col_t[:, mc : mc + 1], scale=1.0,
                        )
                # ---- stage B: gts in [o, n] layout ----
                for ot in range(2):
                    for nh in range(2):
                        ps = mmp.tile([128, MID], f32, tag="ps")
                        p5 = ps[:, :512]
                        for cc in range(2):
                            nc.tensor.matmul(
                                p5,
                                lhsT=wg_t[:, cc, ot * 128 : (ot + 1) * 128],
                                rhs=gtT_t[:, b, cc, nh * 512 : (nh + 1) * 512],
                                start=(cc == 0), stop=(cc == 1),
                            )
                        nc.scalar.activation(
                            out=gts_o[:, b, ot, nh * 512 : (nh + 1) * 512],
                            in_=p5, func=AF.Relu,
                            bias=bgcol_t[:, ot : ot + 1], scale=1.0,
                        )

            for b in range(B_LOC):
                # ---- stage D: o1m^T contraction + z = (o1m - mean)*rstd ----
                for w in range(2):  # waves of 4 i-tiles
                    its = [w * 4 + k for k in range(4)]
                    pss = [mmp.tile([128, MID], f32, name="psw", tag="ps")
                           for _ in its]
                    for jc in range(NT):
                        for k, it in enumerate(its):
                            nc.tensor.matmul(
                                pss[k],
                                lhsT=At[b][jc][:, it * 128 : (it + 1) * 128],
                                rhs=o1t[b][:, jc, :],
                                start=(jc == 0), stop=(jc == NT - 1),
                            )
                    mvw = workp.tile([128, 2, 4], f32, tag="mvw")
                    for k, it in enumerate(its):
                        sv = workp.tile([128, 6], f32, tag="sv")
                        nc.vector.bn_stats(out=sv, in_=pss[k])
                        nc.vector.bn_aggr(out=mvw[:, :, k], in_=sv)
                    stdw = workp.tile([128, 4], f32, tag="stdw")
                    nc.scalar.activation(
                        out=stdw, in_=mvw[:, 1, :], func=AF.Sqrt, bias=eps_t
                    )
                    rstdw = workp.tile([128, 4], f32, tag="rstdw")
                    nc.vector.reciprocal(out=rstdw, in_=stdw)
                    for k, it in enumerate(its):
                        nc.vector.tensor_scalar(
                            out=zt[b][:, it, :], in0=pss[k],
                            scalar1=mvw[:, 0, k : k + 1],
                            scalar2=rstdw[:, k : k + 1],
                            op0=OP.subtract, op1=OP.mult,
                        )

                # ---- stage E: transpose z, fused *g1 + o1mn -> o1n [m, j];
                #      then gconv2 -> o2t [j, o] ----
                for mc in range(4):
                    tp = tpp.tile([128, N], f16, tag="tp")
                    for it in range(NT):
                        nc.tensor.transpose(
                            tp[:, it * 128 : (it + 1) * 128],
                            zt[b][:, it, mc * 128 : (mc + 1) * 128],
                            ident_t,
                        )
                    nc.vector.scalar_tensor_tensor(
                        out=o1n[b][:, mc, :], in0=tp,
                        scalar=g1col_t[:, mc : mc + 1],
                        in1=o1mn[b][:, mc, :], op0=OP.mult, op1=OP.add,
                    )
                    if beta1_nz:
                        nc.gpsimd.tensor_scalar_add(
                            o1n[b][:, mc, :], o1n[b][:, mc, :],
                            beta1col_t[:, mc : mc + 1],
                        )
                for jt in range(NT):
                    ps = mmp.tile([128, MID], f32, tag="ps")
                    p256 = ps[:, :OUT]
                    nc.tensor.matmul(p256, lhsT=ones_t, rhs=b2_t, start=True, stop=False)
                    for mc in range(4):
                        nc.tensor.matmul(
                            p256,
                            lhsT=o1n[b][:, mc, jt * 128 : (jt + 1) * 128],
                            rhs=w2_t[:, mc, :],
                            start=False, stop=(mc == 3),
                        )
                    nc.vector.tensor_scalar_max(o2t[b][:, jt, :], p256, 0.0)

                # ---- stage F: o2m^T contraction + ln2 -> node_feat, output2 ----
                for w in range(2):
                    its = [w * 4 + k for k in range(4)]
                    pss = [mmp.tile([128, MID], f32, name="psw", tag="ps")
                           for _ in its]
                    for jc in range(NT):
                        for k, it in enumerate(its):
                            nc.tensor.matmul(
                                pss[k][:, :OUT],
                                lhsT=At[b][jc][:, it * 128 : (it + 1) * 128],
                                rhs=o2t[b][:, jc, :],
                                start=(jc == 0), stop=(jc == NT - 1),
                            )
                    mvw = workp.tile([128, 2, 4], f32, tag="mvw")
                    for k, it in enumerate(its):
                        sv = workp.tile([128, 6], f32, tag="sv")
                        nc.vector.bn_stats(out=sv, in_=pss[k][:, :OUT])
                        nc.vector.bn_aggr(out=mvw[:, :, k], in_=sv)
                    stdw = workp.tile([128, 4], f32, tag="stdw")
                    nc.scalar.activation(
                        out=stdw, in_=mvw[:, 1, :], func=AF.Sqrt, bias=eps_t
                    )
                    rstdw = workp.tile([128, 4], f32, tag="rstdw")
                    nc.vector.reciprocal(out=rstdw, in_=stdw)
                    for k, it in enumerate(its):
                        z2 = workp.tile([128, OUT], f16, tag="z2")
                        nc.vector.tensor_scalar(
                            out=z2, in0=pss[k][:, :OUT],
                            scalar1=mvw[:, 0, k : k + 1],
                            scalar2=rstdw[:, k : k + 1],
                            op0=OP.subtract, op1=OP.mult,
                        )
                        nf = node_o[:, b, it, :]
                        nc.gpsimd.tensor_tensor(
                            out=nf, in0=z2, in1=g2r16_t, op=OP.mult
                        )
                        if beta2_nz:
                            nc.gpsimd.tensor_tensor(
                                out=nf, in0=nf, in1=beta2_t, op=OP.add
                            )
                        nc.gpsimd.tensor_tensor(
                            out=out2_o[:, b, it, :], in0=nf,
                            in1=o2t[b][:, it, :], op=OP.add,
                        )

            nc.gpsimd.dma_start(
                out=gtsT_d.rearrange("b (o p) n -> p b o n", p=128), in_=gts_o
            )
            nc.gpsimd.dma_start(
                out=node_d.rearrange("b (t p) o -> p b t o", p=128), in_=node_o
            )
            nc.gpsimd.dma_start(
                out=out2_d.rearrange("b (t p) o -> p b t o", p=128), in_=out2_o
            )

    nc.compile()
    return nc


def _compute_col_fast(m1, m2, sm):
    """Exact col == ones proof via a cheap sufficient condition, else None."""
    if m1.min() < 0.0 or m2.min() < 0.0 or sm.min() < 0.0:
        return None
    spos = (sm > 0).astype(F32)
    colnz = np.zeros(N, dtype=bool)
    nz1max = 0.0
    nz2max = 0.0
    for b in range(B):
        p1 = (m1[b] > 0).astype(F32)
        p2 = (m2[b] > 0).astype(F32)
        nz1max = max(nz1max, float((p1 @ spos[b]).max()))
        nz2max = max(nz2max, float((p2 @ spos[b]).max()))
        colnz |= ((p1 + p2).max(axis=0) > 0) & (spos[b] > 0)
    if nz1max <= CHILDS // 4 and nz2max <= CHILDS // 2 and colnz.all():
        return np.ones(N, dtype=F32)
    return None


def _compute_col_slow(m1, m2, sm, li, lj):
    """Exact replica of the reference top-k column-union (numpy)."""
    k4, k2 = CHILDS // 4, CHILDS // 2
    col = np.zeros(N, dtype=bool)
    for b in range(B):
        logits = li[b][:, None] + lj[b][None, :]
        a = 1.0 / (1.0 + np.exp(-logits.astype(F32)))
        mr1 = m1[b] * sm[b][None, :]
        mr2 = m2[b] * sm[b][None, :]
        a1 = a * mr1
        a2 = a * mr2
        # lax.top_k ties -> lowest index; stable argsort on (-a) reproduces it.
        col[np.argsort(-a1, axis=1, kind="stable")[:, :k4].ravel()] = True
        col[np.argsort(a1, axis=1, kind="stable")[:, :k4].ravel()] = True
        col[np.argsort(-a2, axis=1, kind="stable")[:, :k2].ravel()] = True
        col[np.argsort(a2, axis=1, kind="stable")[:, :k4].ravel()] = True
    return col.astype(F32)


def kernel(**inputs):
    x = np.ascontiguousarray(np.asarray(inputs["x"], dtype=F32))
    m1 = np.asarray(inputs["masks_roi1"], dtype=F32)
    m2 = np.asarray(inputs["masks_roi2"], dtype=F32)
    sm = np.asarray(inputs["score_mask"], dtype=F32)
    gt = np.asarray(inputs["gt_feat"], dtype=F32)
    W_att = np.asarray(inputs["W_att"], dtype=F32)
    b_att = np.asarray(inputs["b_att"], dtype=F32)
    W1 = np.asarray(inputs["W1"], dtype=F32)
    b1 = np.asarray(inputs["b1"], dtype=F32)
    W2 = np.asarray(inputs["W2"], dtype=F32)
    b2 = np.asarray(inputs["b2"], dtype=F32)
    g1 = np.asarray(inputs["g1"], dtype=F32)
    beta1 = np.asarray(inputs["beta1"], dtype=F32)
    g2 = np.asarray(inputs["g2"], dtype=F32)
    beta2 = np.asarray(inputs["beta2"], dtype=F32)
    Wg = np.asarray(inputs["Wg"], dtype=F32)
    bg = np.asarray(inputs["bg"], dtype=F32)

    assert x.shape == (B, N, CIN) and W_att.shape == (2 * CIN, 1)

    # ---- host prep: tiny vector math + layout/dtype staging ----
    lj = x.reshape(B * N, CIN) @ W_att[:CIN, 0]
    lj = lj.reshape(B, N)
    li = x.reshape(B * N, CIN) @ W_att[CIN:, 0]
    li = li.reshape(B, N) + b_att[0]

    col = _compute_col_fast(m1, m2, sm)
    if col is None:
        col = _compute_col_slow(m1, m2, sm, li, lj)

    # One fused mask tensor: msT[j, i] = (m1+m2)[i, j] * score[j] * col[j],
    # with the f_diag term folded exactly onto the diagonal as
    # f[j] / sigmoid(l_jj)   (then the device's sigmoid multiply restores f).
    msT = (m1 + m2).transpose(0, 2, 1) * (sm * col[None, :])[:, :, None]
    f = (sm == 0).astype(F32)
    ldiag = li + lj  # l_jj = li[j] + lj[j]
    sig_diag = 1.0 / (1.0 + np.exp(-ldiag))
    didx = np.arange(N)
    msT[:, didx, didx] += f / sig_diag
    msT = np.ascontiguousarray(msT).astype(F16)

    xT = np.ascontiguousarray(x.transpose(0, 2, 1)).astype(F16)
    gtT = np.ascontiguousarray(gt.transpose(0, 2, 1)).astype(F16)
    lirow = li.astype(F16)
    ljT = np.ascontiguousarray(lj.reshape(B, NT, 128).transpose(0, 2, 1)).astype(F32)

    # Weights: block-diagonal transposed layouts for the grouped convs.
    w1bd = np.zeros((CIN, MID), dtype=F32)
    for g in range(G):
        w1bd[64 * g : 64 * (g + 1), 128 * g : 128 * (g + 1)] = W1[
            128 * g : 128 * (g + 1), :
        ].T
    w2bd = np.zeros((MID, OUT), dtype=F32)
    for g in range(G):
        w2bd[128 * g : 128 * (g + 1), 64 * g : 64 * (g + 1)] = W2[
            64 * g : 64 * (g + 1), :
        ].T
    wgK = np.ascontiguousarray(Wg.reshape(2, 128, OUT)).astype(F16)
    w1K = np.ascontiguousarray(w1bd.reshape(2, 128, MID)).astype(F16)
    w2K = np.ascontiguousarray(w2bd.reshape(4, 128, OUT)).astype(F16)

    shared = {
        "wgK": wgK,
        "w1K": w1K,
        "w2K": w2K,
        "bgcol": np.ascontiguousarray(bg.reshape(2, 128)).astype(F32),
        "b1col": np.ascontiguousarray(b1.reshape(4, 128)).astype(F32),
        "g1col": np.ascontiguousarray(g1.reshape(4, 128)).astype(F32),
        "b1row": b1.reshape(1, MID).astype(F16),
        "b2row": b2.reshape(1, OUT).astype(F16),
        "g2row16": g2.reshape(1, OUT).astype(F16),
        "beta1col": np.ascontiguousarray(beta1.reshape(4, 128)).astype(F32),
        "beta2row": beta2.reshape(1, OUT).astype(F16),
        "ident": np.eye(128, dtype=F16),
        "onescol": np.ones((1, 128), dtype=F16),
    }
    in_maps = []
    for c in range(NCORES):
        s = slice(B_LOC * c, B_LOC * (c + 1))
        in_maps.append(
            {
                "msT": msT[s],
                "xT": xT[s],
                "gtT": gtT[s],
                "lirow": lirow[s],
                "ljT": ljT[s],
                **shared,
            }
        )

    beta_key = (bool(np.any(beta1)), bool(np.any(beta2)))
    if beta_key not in _PROGRAM_CACHE:
        _PROGRAM_CACHE[beta_key] = _build_program(*beta_key)
    nc = _PROGRAM_CACHE[beta_key]

    global _LAST_IN_MAPS
    _LAST_IN_MAPS = in_maps

    from concourse.bass_utils import run_bass_kernel_spmd

    res = run_bass_kernel_spmd(nc, in_maps, core_ids=list(range(NCORES)))
    results = res.results if hasattr(res, "results") else res

    gts = np.concatenate(
        [results[c]["gtsT"].transpose(0, 2, 1) for c in range(NCORES)], axis=0
    )
    node_feat = np.concatenate([results[c]["node"] for c in range(NCORES)], axis=0)
    output2 = np.concatenate([results[c]["out2"] for c in range(NCORES)], axis=0)
    return output2.astype(F32), gts.astype(F32), node_feat.astype(F32)


# revision 13
# speedup vs baseline: 2.2588x; 1.2770x over previous
"""Trainium2 Bass kernel for nn_Graph_module_net_0_loss_type_18631568130084.

GNN message-passing block:
  gts       = relu(gt_feat @ Wg + bg)
  attn[i,j] = sigmoid(x[j]@Wq + x[i]@Wk + b_att)          (H == 1)
  atten     = (attn * (mr1+mr2) * col + f_diag) / CHILDS  ([B,H,Nj,Ni])
  o1 = relu(gconv1(x^T)); o1 += ln1(o1 @ atten)^T
  o2 = relu(gconv2(o1));  node_feat = ln2(o2 @ atten);  output2 = (o2 + node_feat^T)^T

Sharding: data-parallel over batch B=16 -> 2 batches per core on 8 cores.

v3 design (v1 baseline 180us, v2 322us - gpsimd tensor ops are ~3-8x slower
than DVE, so v3 minimizes elementwise passes and keeps PSUM work on DVE/ACT):
 * ONE mask tensor: host pre-folds (m1+m2)*score*col into msT fp16 and folds
   the f_diag term exactly onto the diagonal as f[j]/sigmoid(l_jj); device
   atten^T = sigmoid_tile * msT_tile (one gpsimd TT per j-tile).
 * gts in [OUT, N] layout: bg is a per-partition bias fused into ACT relu.
 * gconv1 computed in BOTH layouts: o1t [j,m] (stage-D rhs) and o1mn [m,n]
   (per-partition bias b1 free in ACT relu).  LN1 apply is split:
   z = (o1m - mean)*rstd (one DVE tensor_scalar, 2 scalar operands), PE
   transposes z, and o1_new^T[m,j] = z^T * g1[m] + o1mn is ONE DVE
   scalar_tensor_tensor from PSUM - the gamma-multiply, residual add and
   PSUM->SBUF copy all fused into the transpose epilogue.
 * LN rstd: variances of 4 i-tiles packed into [128,4], one ACT Sqrt
   (bias=eps) + one DVE reciprocal per wave.  All sigmoids (both batches)
   run before any sqrt => exactly 2 ACT table loads.
 * DMA: few large transfers; f16 consts + x on sync ahead of masks,
   lirow/ljT on the ACT queue so sigmoids start ~2us in; outputs fp16 on
   gpsimd at the end (host casts back to fp32 / un-transposes gts).
 * Stage D matmuls issue jc-outer in waves of 4 i-tiles so PE starts
   contracting as soon as At[jc] is ready; 1/CHILDS cancels in both
   layernorms (eps rescaled by CHILDS^2).
 * The top-k "col" mask is computed exactly on the host: a cheap sufficient
   condition proves col == all-ones; otherwise an exact numpy replica runs.
"""

import numpy as np

B = 16
N = 1024
CIN = 256
MID = 512
OUT = 256
G = 4
CHILDS = 512
NCORES = 8
B_LOC = B // NCORES  # 2
NT = N // 128  # 8
EPS_LN = 1e-6 * float(CHILDS) ** 2  # eps rescaled because we drop the 1/CHILDS

F16 = np.float16
F32 = np.float32

_PROGRAM_CACHE = {}


def _build_program(beta1_nz: bool, beta2_nz: bool):
    import concourse.bacc as bacc
    import concourse.tile as tile
    from concourse import mybir

    f16 = mybir.dt.float16
    f32 = mybir.dt.float32
    AF = mybir.ActivationFunctionType
    OP = mybir.AluOpType

    nc = bacc.Bacc("TRN2", debug=False)

    def din(name, shape, dt):
        return nc.dram_tensor(name, shape, dt, kind="ExternalInput").ap()

    def dout(name, shape, dt):
        return nc.dram_tensor(name, shape, dt, kind="ExternalOutput").ap()

    # Per-core inputs (leading dim B_LOC where batch-dependent).
    msT_d = din("msT", [B_LOC, N, N], f16)       # (m1+m2)*score*col (+diag) ^T
    xT_d = din("xT", [B_LOC, CIN, N], f16)       # x^T   [c, n]
    gtT_d = din("gtT", [B_LOC, CIN, N], f16)     # gt^T  [c, n]
    lirow_d = din("lirow", [B_LOC, N], f16)      # x@Wk + b_att      (per-i row)
    ljT_d = din("ljT", [B_LOC, 128, NT], f32)    # x@Wq chunked      (per-j bias)
    # Replicated weights.
    wg_d = din("wgK", [2, 128, OUT], f16)        # Wg   (c-chunks)
    w1_d = din("w1K", [2, 128, MID], f16)        # block-diag W1^T (c-chunks)
    w2_d = din("w2K", [4, 128, OUT], f16)        # block-diag W2^T (m-chunks)
    bgcol_d = din("bgcol", [2, 128], f32)        # bg per o-tile (per-partition)
    b1col_d = din("b1col", [4, 128], f32)        # b1 per m-chunk (per-partition)
    g1col_d = din("g1col", [4, 128], f32)        # g1 per m-chunk (per-partition)
    b1_d = din("b1row", [1, MID], f16)
    b2_d = din("b2row", [1, OUT], f16)
    g2r16_d = din("g2row16", [1, OUT], f16)
    beta1col_d = din("beta1col", [4, 128], f32)
    beta2_d = din("beta2row", [1, OUT], f16)
    ident_d = din("ident", [128, 128], f16)
    ones_d = din("onescol", [1, 128], f16)

    gtsT_d = dout("gtsT", [B_LOC, OUT, N], f16)  # [o, n] - host un-transposes
    node_d = dout("node", [B_LOC, N, OUT], f16)
    out2_d = dout("out2", [B_LOC, N, OUT], f16)

    with tile.TileContext(nc) as tc:
        with tc.tile_pool(name="const", bufs=1) as constp, \
             tc.tile_pool(name="inp", bufs=1) as inp, \
             tc.tile_pool(name="at", bufs=1) as atp, \
             tc.tile_pool(name="big", bufs=1) as bigp, \
             tc.tile_pool(name="work", bufs=4) as workp, \
             tc.tile_pool(name="sg", bufs=3) as sgp, \
             tc.tile_pool(name="outs", bufs=1) as outp, \
             tc.tile_pool(name="mm", bufs=6, space="PSUM") as mmp, \
             tc.tile_pool(name="tp", bufs=2, space="PSUM") as tpp:

            # ---- f16 weights + x early on sync (tiny transfers, needed by
            # B/C within ~10us) ----
            ident_t = constp.tile([128, 128], f16)
            nc.sync.dma_start(out=ident_t, in_=ident_d)
            ones_t = constp.tile([1, 128], f16)
            nc.sync.dma_start(out=ones_t, in_=ones_d)
            wg_t = constp.tile([128, 2, OUT], f16)
            nc.sync.dma_start(out=wg_t, in_=wg_d.rearrange("c p f -> p c f"))
            w1_t = constp.tile([128, 2, MID], f16)
            nc.sync.dma_start(out=w1_t, in_=w1_d.rearrange("c p f -> p c f"))
            w2_t = constp.tile([128, 4, OUT], f16)
            nc.sync.dma_start(out=w2_t, in_=w2_d.rearrange("c p f -> p c f"))
            b1_t = constp.tile([1, MID], f16)
            nc.sync.dma_start(out=b1_t, in_=b1_d)
            b2_t = constp.tile([1, OUT], f16)
            nc.sync.dma_start(out=b2_t, in_=b2_d)
            xT_t = inp.tile([128, B_LOC, 2, N], f16)
            nc.sync.dma_start(
                out=xT_t, in_=xT_d.rearrange("b (c p) n -> p b c n", p=128)
            )

            # masks batch-0 first, then gt (needed ~15us in), then batch-1
            At = [
                [atp.tile([128, N], f16, name=f"At{b}_{jt}", tag=f"At{b}_{jt}")
                 for jt in range(NT)]
                for b in range(B_LOC)
            ]
            for jt in range(NT):
                nc.sync.dma_start(
                    out=At[0][jt], in_=msT_d[0, jt * 128 : (jt + 1) * 128, :]
                )
            gtT_t = inp.tile([128, B_LOC, 2, N], f16)
            nc.sync.dma_start(
                out=gtT_t, in_=gtT_d.rearrange("b (c p) n -> p b c n", p=128)
            )
            for jt in range(NT):
                nc.sync.dma_start(
                    out=At[1][jt], in_=msT_d[1, jt * 128 : (jt + 1) * 128, :]
                )

            # ---- ACT queue: lirow/ljT first so sigmoids start immediately ----
            lirow_t = inp.tile([128, B_LOC, N], f16)
            nc.scalar.dma_start(
                out=lirow_t, in_=lirow_d[None].to_broadcast([128, B_LOC, N])
            )
            ljT_t = inp.tile([128, B_LOC, NT], f32)
            nc.scalar.dma_start(out=ljT_t, in_=ljT_d.rearrange("b p t -> p b t"))

            # ---- f32 / late-needed consts on gpsimd (idle until ~12us) ----
            bgcol_t = constp.tile([128, 2], f32)
            nc.gpsimd.dma_start(out=bgcol_t, in_=bgcol_d.rearrange("o p -> p o"))
            b1col_t = constp.tile([128, 4], f32)
            nc.gpsimd.dma_start(out=b1col_t, in_=b1col_d.rearrange("c p -> p c"))
            g1col_t = constp.tile([128, 4], f32)
            nc.gpsimd.dma_start(out=g1col_t, in_=g1col_d.rearrange("c p -> p c"))
            g2r16_t = constp.tile([128, OUT], f16)
            nc.gpsimd.dma_start(out=g2r16_t, in_=g2r16_d.to_broadcast([128, OUT]))
            if beta1_nz:
                beta1col_t = constp.tile([128, 4], f32)
                nc.gpsimd.dma_start(
                    out=beta1col_t, in_=beta1col_d.rearrange("c p -> p c")
                )
            if beta2_nz:
                beta2_t = constp.tile([128, OUT], f16)
                nc.gpsimd.dma_start(out=beta2_t, in_=beta2_d.to_broadcast([128, OUT]))
            eps_t = constp.tile([128, 1], f32)
            nc.vector.memset(eps_t, EPS_LN)

            # Per-batch activation tensors (both batches resident).
            o1t = [bigp.tile([128, NT, MID], f16, name=f"o1t{b}", tag=f"o1t{b}")
                   for b in range(B_LOC)]
            o1mn = [bigp.tile([128, 4, N], f16, name=f"o1mn{b}", tag=f"o1mn{b}")
                    for b in range(B_LOC)]
            zt = [bigp.tile([128, NT, MID], f16, name=f"zt{b}", tag=f"zt{b}")
                  for b in range(B_LOC)]
            o1n = [bigp.tile([128, 4, N], f16, name=f"o1n{b}", tag=f"o1n{b}")
                   for b in range(B_LOC)]
            o2t = [bigp.tile([128, NT, OUT], f16, name=f"o2t{b}", tag=f"o2t{b}")
                   for b in range(B_LOC)]

            gts_o = outp.tile([128, B_LOC, 2, N], f16)
            node_o = outp.tile([128, B_LOC, NT, OUT], f16)
            out2_o = outp.tile([128, B_LOC, NT, OUT], f16)

            def stage_A(b):
                for jt in range(NT):
                    sg = sgp.tile([128, N], f16, name="sg", tag="sg")
                    nc.scalar.activation(
                        out=sg, in_=lirow_t[:, b, :], func=AF.Sigmoid,
                        bias=ljT_t[:, b, jt : jt + 1], scale=1.0,
                    )
                    nc.vector.tensor_tensor(
                        out=At[b][jt], in0=At[b][jt], in1=sg, op=OP.mult
                    )

            def stage_C(b):
                # gconv1 -> o1t [j, m]
                for jt in range(NT):
                    ps = mmp.tile([128, MID], f32, name="ps", tag="ps")
                    nc.tensor.matmul(ps, lhsT=ones_t, rhs=b1_t, start=True, stop=False)
                    for cc in range(2):
                        nc.tensor.matmul(
                            ps,
                            lhsT=xT_t[:, b, cc, jt * 128 : (jt + 1) * 128],
                            rhs=w1_t[:, cc, :],
                            start=False, stop=(cc == 1),
                        )
                    nc.scalar.activation(out=o1t[b][:, jt, :], in_=ps, func=AF.Relu)
                # gconv1 -> o1mn [m, n] (bias per-partition)
                for mc in range(4):
                    for nh in range(2):
                        ps = mmp.tile([128, MID], f32, name="ps", tag="ps")
                        for cc in range(2):
                            nc.tensor.matmul(
                                ps,
                                lhsT=w1_t[:, cc, mc * 128 : (mc + 1) * 128],
                                rhs=xT_t[:, b, cc, nh * 512 : (nh + 1) * 512],
                                start=(cc == 0), stop=(cc == 1),
                            )
                        nc.scalar.activation(
                            out=o1mn[b][:, mc, nh * 512 : (nh + 1) * 512],
                            in_=ps, func=AF.Relu,
                            bias=b1col_t[:, mc : mc + 1], scale=1.0,
                        )

            def stage_B(b):
                # gts in [o, n] layout
                for ot in range(2):
                    for nh in range(2):
                        ps = mmp.tile([128, MID], f32, name="ps", tag="ps")
                        p5 = ps[:, :512]
                        for cc in range(2):
                            nc.tensor.matmul(
                                p5,
                                lhsT=wg_t[:, cc, ot * 128 : (ot + 1) * 128],
                                rhs=gtT_t[:, b, cc, nh * 512 : (nh + 1) * 512],
                                start=(cc == 0), stop=(cc == 1),
                            )
                        nc.scalar.activation(
                            out=gts_o[:, b, ot, nh * 512 : (nh + 1) * 512],
                            in_=p5, func=AF.Relu,
                            bias=bgcol_t[:, ot : ot + 1], scale=1.0,
                        )

            def stage_D(b):
                # o1m^T contraction + z = (o1m - mean)*rstd
                for w in range(2):  # waves of 4 i-tiles
                    its = [w * 4 + k for k in range(4)]
                    pss = [mmp.tile([128, MID], f32, name="psw", tag="ps")
                           for _ in its]
                    for jc in range(NT):
                        for k, it in enumerate(its):
                            nc.tensor.matmul(
                                pss[k],
                                lhsT=At[b][jc][:, it * 128 : (it + 1) * 128],
                                rhs=o1t[b][:, jc, :],
                                start=(jc == 0), stop=(jc == NT - 1),
                            )
                    mvw = workp.tile([128, 2, 4], f32, name="mvw", tag="mvw")
                    for k, it in enumerate(its):
                        sv = workp.tile([128, 6], f32, name="sv", tag="sv")
                        nc.vector.bn_stats(out=sv, in_=pss[k])
                        nc.vector.bn_aggr(out=mvw[:, :, k], in_=sv)
                    stdw = workp.tile([128, 4], f32, name="stdw", tag="stdw")
                    nc.scalar.activation(
                        out=stdw, in_=mvw[:, 1, :], func=AF.Sqrt, bias=eps_t
                    )
                    rstdw = workp.tile([128, 4], f32, name="rstdw", tag="rstdw")
                    nc.vector.reciprocal(out=rstdw, in_=stdw)
                    for k, it in enumerate(its):
                        nc.vector.tensor_scalar(
                            out=zt[b][:, it, :], in0=pss[k],
                            scalar1=mvw[:, 0, k : k + 1],
                            scalar2=rstdw[:, k : k + 1],
                            op0=OP.subtract, op1=OP.mult,
                        )

            def stage_E(b):
                # transpose z, fused *g1 + o1mn -> o1n [m, j]; gconv2 -> o2t
                for mc in range(4):
                    tp = tpp.tile([128, N], f16, name="tp", tag="tp")
                    for it in range(NT):
                        nc.tensor.transpose(
                            tp[:, it * 128 : (it + 1) * 128],
                            zt[b][:, it, mc * 128 : (mc + 1) * 128],
                            ident_t,
                        )
                    nc.vector.scalar_tensor_tensor(
                        out=o1n[b][:, mc, :], in0=tp,
                        scalar=g1col_t[:, mc : mc + 1],
                        in1=o1mn[b][:, mc, :], op0=OP.mult, op1=OP.add,
                    )
                    if beta1_nz:
                        nc.gpsimd.tensor_scalar_add(
                            o1n[b][:, mc, :], o1n[b][:, mc, :],
                            beta1col_t[:, mc : mc + 1],
                        )
                for jt in range(NT):
                    ps = mmp.tile([128, MID], f32, name="ps", tag="ps")
                    p256 = ps[:, :OUT]
                    nc.tensor.matmul(p256, lhsT=ones_t, rhs=b2_t, start=True, stop=False)
                    for mc in range(4):
                        nc.tensor.matmul(
                            p256,
                            lhsT=o1n[b][:, mc, jt * 128 : (jt + 1) * 128],
                            rhs=w2_t[:, mc, :],
                            start=False, stop=(mc == 3),
                        )
                    nc.vector.tensor_scalar_max(o2t[b][:, jt, :], p256, 0.0)

            def stage_F(b, eng):
                # o2m^T contraction + ln2 -> node_feat, output2.  The nf/out2
                # elementwise tail goes on `eng`: gpsimd for batch 0 (overlaps
                # batch 1 PE work), vector for batch 1 (shortest kernel tail).
                for w in range(2):
                    its = [w * 4 + k for k in range(4)]
                    pss = [mmp.tile([128, MID], f32, name="psw", tag="ps")
                           for _ in its]
                    for jc in range(NT):
                        for k, it in enumerate(its):
                            nc.tensor.matmul(
                                pss[k][:, :OUT],
                                lhsT=At[b][jc][:, it * 128 : (it + 1) * 128],
                                rhs=o2t[b][:, jc, :],
                                start=(jc == 0), stop=(jc == NT - 1),
                            )
                    mvw = workp.tile([128, 2, 4], f32, name="mvw", tag="mvw")
                    for k, it in enumerate(its):
                        sv = workp.tile([128, 6], f32, name="sv", tag="sv")
                        nc.vector.bn_stats(out=sv, in_=pss[k][:, :OUT])
                        nc.vector.bn_aggr(out=mvw[:, :, k], in_=sv)
                    stdw = workp.tile([128, 4], f32, name="stdw", tag="stdw")
                    nc.scalar.activation(
                        out=stdw, in_=mvw[:, 1, :], func=AF.Sqrt, bias=eps_t
                    )
                    rstdw = workp.tile([128, 4], f32, name="rstdw", tag="rstdw")
                    nc.vector.reciprocal(out=rstdw, in_=stdw)
                    for k, it in enumerate(its):
                        z2 = workp.tile([128, OUT], f16, name="z2", tag="z2")
                        nc.vector.tensor_scalar(
                            out=z2, in0=pss[k][:, :OUT],
                            scalar1=mvw[:, 0, k : k + 1],
                            scalar2=rstdw[:, k : k + 1],
                            op0=OP.subtract, op1=OP.mult,
                        )
                        nf = node_o[:, b, it, :]
                        eng.tensor_tensor(out=nf, in0=z2, in1=g2r16_t, op=OP.mult)
                        if beta2_nz:
                            eng.tensor_tensor(out=nf, in0=nf, in1=beta2_t, op=OP.add)
                        eng.tensor_tensor(
                            out=out2_o[:, b, it, :], in0=nf,
                            in1=o2t[b][:, it, :], op=OP.add,
                        )

            stage_A(0)
            stage_C(0)
            stage_B(0)
            stage_D(0)
            stage_A(1)
            stage_C(1)
            stage_B(1)
            stage_E(0)
            stage_F(0, nc.gpsimd)
            stage_D(1)
            stage_E(1)
            stage_F(1, nc.vector)

            nc.gpsimd.dma_start(
                out=gtsT_d.rearrange("b (o p) n -> p b o n", p=128), in_=gts_o
            )
            nc.gpsimd.dma_start(
                out=node_d.rearrange("b (t p) o -> p b t o", p=128), in_=node_o
            )
            nc.gpsimd.dma_start(
                out=out2_d.rearrange("b (t p) o -> p b t o", p=128), in_=out2_o
            )

    nc.compile()
    return nc


def _compute_col_fast(m1, m2, sm):
    """Exact col == ones proof via a cheap sufficient condition, else None."""
    if m1.min() < 0.0 or m2.min() < 0.0 or sm.min() < 0.0:
        return None
    spos = (sm > 0).astype(F32)
    colnz = np.zeros(N, dtype=bool)
    nz1max = 0.0
    nz2max = 0.0
    for b in range(B):
        p1 = (m1[b] > 0).astype(F32)
        p2 = (m2[b] > 0).astype(F32)
        nz1max = max(nz1max, float((p1 @ spos[b]).max()))
        nz2max = max(nz2max, float((p2 @ spos[b]).max()))
        colnz |= ((p1 + p2).max(axis=0) > 0) & (spos[b] > 0)
    if nz1max <= CHILDS // 4 and nz2max <= CHILDS // 2 and colnz.all():
        return np.ones(N, dtype=F32)
    return None


def _compute_col_slow(m1, m2, sm, li, lj):
    """Exact replica of the reference top-k column-union (numpy)."""
    k4, k2 = CHILDS // 4, CHILDS // 2
    col = np.zeros(N, dtype=bool)
    for b in range(B):
        logits = li[b][:, None] + lj[b][None, :]
        a = 1.0 / (1.0 + np.exp(-logits.astype(F32)))
        mr1 = m1[b] * sm[b][None, :]
        mr2 = m2[b] * sm[b][None, :]
        a1 = a * mr1
        a2 = a * mr2
        # lax.top_k ties -> lowest index; stable argsort on (-a) reproduces it.
        col[np.argsort(-a1, axis=1, kind="stable")[:, :k4].ravel()] = True
        col[np.argsort(a1, axis=1, kind="stable")[:, :k4].ravel()] = True
        col[np.argsort(-a2, axis=1, kind="stable")[:, :k2].ravel()] = True
        col[np.argsort(a2, axis=1, kind="stable")[:, :k4].ravel()] = True
    return col.astype(F32)


def kernel(**inputs):
    x = np.ascontiguousarray(np.asarray(inputs["x"], dtype=F32))
    m1 = np.asarray(inputs["masks_roi1"], dtype=F32)
    m2 = np.asarray(inputs["masks_roi2"], dtype=F32)
    sm = np.asarray(inputs["score_mask"], dtype=F32)
    gt = np.asarray(inputs["gt_feat"], dtype=F32)
    W_att = np.asarray(inputs["W_att"], dtype=F32)
    b_att = np.asarray(inputs["b_att"], dtype=F32)
    W1 = np.asarray(inputs["W1"], dtype=F32)
    b1 = np.asarray(inputs["b1"], dtype=F32)
    W2 = np.asarray(inputs["W2"], dtype=F32)
    b2 = np.asarray(inputs["b2"], dtype=F32)
    g1 = np.asarray(inputs["g1"], dtype=F32)
    beta1 = np.asarray(inputs["beta1"], dtype=F32)
    g2 = np.asarray(inputs["g2"], dtype=F32)
    beta2 = np.asarray(inputs["beta2"], dtype=F32)
    Wg = np.asarray(inputs["Wg"], dtype=F32)
    bg = np.asarray(inputs["bg"], dtype=F32)

    assert x.shape == (B, N, CIN) and W_att.shape == (2 * CIN, 1)

    # ---- host prep: tiny vector math + layout/dtype staging ----
    lj = x.reshape(B * N, CIN) @ W_att[:CIN, 0]
    lj = lj.reshape(B, N)
    li = x.reshape(B * N, CIN) @ W_att[CIN:, 0]
    li = li.reshape(B, N) + b_att[0]

    col = _compute_col_fast(m1, m2, sm)
    if col is None:
        col = _compute_col_slow(m1, m2, sm, li, lj)

    # One fused mask tensor: msT[j, i] = (m1+m2)[i, j] * score[j] * col[j],
    # with the f_diag term folded exactly onto the diagonal as
    # f[j] / sigmoid(l_jj)   (then the device's sigmoid multiply restores f).
    msT = (m1 + m2).transpose(0, 2, 1) * (sm * col[None, :])[:, :, None]
    f = (sm == 0).astype(F32)
    ldiag = li + lj  # l_jj = li[j] + lj[j]
    sig_diag = 1.0 / (1.0 + np.exp(-ldiag))
    didx = np.arange(N)
    msT[:, didx, didx] += f / sig_diag
    msT = np.ascontiguousarray(msT).astype(F16)

    xT = np.ascontiguousarray(x.transpose(0, 2, 1)).astype(F16)
    gtT = np.ascontiguousarray(gt.transpose(0, 2, 1)).astype(F16)
    lirow = li.astype(F16)
    ljT = np.ascontiguousarray(lj.reshape(B, NT, 128).transpose(0, 2, 1)).astype(F32)

    # Weights: block-diagonal transposed layouts for the grouped convs.
    w1bd = np.zeros((CIN, MID), dtype=F32)
    for g in range(G):
        w1bd[64 * g : 64 * (g + 1), 128 * g : 128 * (g + 1)] = W1[
            128 * g : 128 * (g + 1), :
        ].T
    w2bd = np.zeros((MID, OUT), dtype=F32)
    for g in range(G):
        w2bd[128 * g : 128 * (g + 1), 64 * g : 64 * (g + 1)] = W2[
            64 * g : 64 * (g + 1), :
        ].T
    wgK = np.ascontiguousarray(Wg.reshape(2, 128, OUT)).astype(F16)
    w1K = np.ascontiguousarray(w1bd.reshape(2, 128, MID)).astype(F16)
    w2K = np.ascontiguousarray(w2bd.reshape(4, 128, OUT)).astype(F16)

    shared = {
        "wgK": wgK,
        "w1K": w1K,
        "w2K": w2K,
        "bgcol": np.ascontiguousarray(bg.reshape(2, 128)).astype(F32),
        "b1col": np.ascontiguousarray(b1.reshape(4, 128)).astype(F32),
        "g1col": np.ascontiguousarray(g1.reshape(4, 128)).astype(F32),
        "b1row": b1.reshape(1, MID).astype(F16),
        "b2row": b2.reshape(1, OUT).astype(F16),
        "g2row16": g2.reshape(1, OUT).astype(F16),
        "beta1col": np.ascontiguousarray(beta1.reshape(4, 128)).astype(F32),
        "beta2row": beta2.reshape(1, OUT).astype(F16),
        "ident": np.eye(128, dtype=F16),
        "onescol": np.ones((1, 128), dtype=F16),
    }
    in_maps = []
    for c in range(NCORES):
        s = slice(B_LOC * c, B_LOC * (c + 1))
        in_maps.append(
            {
                "msT": msT[s],
                "xT": xT[s],
                "gtT": gtT[s],
                "lirow": lirow[s],
                "ljT": ljT[s],
                **shared,
            }
        )

    beta_key = (bool(np.any(beta1)), bool(np.any(beta2)))
    if beta_key not in _PROGRAM_CACHE:
        _PROGRAM_CACHE[beta_key] = _build_program(*beta_key)
    nc = _PROGRAM_CACHE[beta_key]

    global _LAST_IN_MAPS
    _LAST_IN_MAPS = in_maps

    from concourse.bass_utils import run_bass_kernel_spmd

    res = run_bass_kernel_spmd(nc, in_maps, core_ids=list(range(NCORES)))
    results = res.results if hasattr(res, "results") else res

    gts = np.concatenate(
        [results[c]["gtsT"].transpose(0, 2, 1) for c in range(NCORES)], axis=0
    )
    node_feat = np.concatenate([results[c]["node"] for c in range(NCORES)], axis=0)
    output2 = np.concatenate([results[c]["out2"] for c in range(NCORES)], axis=0)
    return output2.astype(F32), gts.astype(F32), node_feat.astype(F32)


# revision 14
# speedup vs baseline: 2.3383x; 1.0352x over previous
"""Trainium2 Bass kernel for nn_Graph_module_net_0_loss_type_18631568130084.

GNN message-passing block:
  gts       = relu(gt_feat @ Wg + bg)
  attn[i,j] = sigmoid(x[j]@Wq + x[i]@Wk + b_att)          (H == 1)
  atten     = (attn * (mr1+mr2) * col + f_diag) / CHILDS  ([B,H,Nj,Ni])
  o1 = relu(gconv1(x^T)); o1 += ln1(o1 @ atten)^T
  o2 = relu(gconv2(o1));  node_feat = ln2(o2 @ atten);  output2 = (o2 + node_feat^T)^T

Sharding: data-parallel over batch B=16 -> 2 batches per core on 8 cores.

v3 design (v1 baseline 180us, v2 322us - gpsimd tensor ops are ~3-8x slower
than DVE, so v3 minimizes elementwise passes and keeps PSUM work on DVE/ACT):
 * ONE mask tensor: host pre-folds (m1+m2)*score*col into msT fp16 and folds
   the f_diag term exactly onto the diagonal as f[j]/sigmoid(l_jj); device
   atten^T = sigmoid_tile * msT_tile (one gpsimd TT per j-tile).
 * gts in [OUT, N] layout: bg is a per-partition bias fused into ACT relu.
 * gconv1 computed in BOTH layouts: o1t [j,m] (stage-D rhs) and o1mn [m,n]
   (per-partition bias b1 free in ACT relu).  LN1 apply is split:
   z = (o1m - mean)*rstd (one DVE tensor_scalar, 2 scalar operands), PE
   transposes z, and o1_new^T[m,j] = z^T * g1[m] + o1mn is ONE DVE
   scalar_tensor_tensor from PSUM - the gamma-multiply, residual add and
   PSUM->SBUF copy all fused into the transpose epilogue.
 * LN rstd: variances of 4 i-tiles packed into [128,4], one ACT Sqrt
   (bias=eps) + one DVE reciprocal per wave.  All sigmoids (both batches)
   run before any sqrt => exactly 2 ACT table loads.
 * DMA: few large transfers; f16 consts + x on sync ahead of masks,
   lirow/ljT on the ACT queue so sigmoids start ~2us in; outputs fp16 on
   gpsimd at the end (host casts back to fp32 / un-transposes gts).
 * Stage D matmuls issue jc-outer in waves of 4 i-tiles so PE starts
   contracting as soon as At[jc] is ready; 1/CHILDS cancels in both
   layernorms (eps rescaled by CHILDS^2).
 * The top-k "col" mask is computed exactly on the host: a cheap sufficient
   condition proves col == all-ones; otherwise an exact numpy replica runs.
"""

import numpy as np

B = 16
N = 1024
CIN = 256
MID = 512
OUT = 256
G = 4
CHILDS = 512
NCORES = 8
B_LOC = B // NCORES  # 2
NT = N // 128  # 8
EPS_LN = 1e-6 * float(CHILDS) ** 2  # eps rescaled because we drop the 1/CHILDS

F16 = np.float16
F32 = np.float32

_PROGRAM_CACHE = {}


def _build_program(beta1_nz: bool, beta2_nz: bool):
    import concourse.bacc as bacc
    import concourse.tile as tile
    from concourse import mybir

    f16 = mybir.dt.float16
    f32 = mybir.dt.float32
    AF = mybir.ActivationFunctionType
    OP = mybir.AluOpType

    nc = bacc.Bacc("TRN2", debug=False)

    def din(name, shape, dt):
        return nc.dram_tensor(name, shape, dt, kind="ExternalInput").ap()

    def dout(name, shape, dt):
        return nc.dram_tensor(name, shape, dt, kind="ExternalOutput").ap()

    # Per-core inputs (leading dim B_LOC where batch-dependent).
    msT_d = din("msT", [B_LOC, N, N], f16)       # (m1+m2)*score*col (+diag) ^T
    xT_d = din("xT", [B_LOC, CIN, N], f16)       # x^T   [c, n]
    gtT_d = din("gtT", [B_LOC, CIN, N], f16)     # gt^T  [c, n]
    lirow_d = din("lirow", [B_LOC, N], f16)      # x@Wk + b_att      (per-i row)
    ljT_d = din("ljT", [B_LOC, 128, NT], f32)    # x@Wq chunked      (per-j bias)
    # Replicated weights.
    wg_d = din("wgK", [2, 128, OUT], f16)        # Wg   (c-chunks)
    w1_d = din("w1K", [2, 128, MID], f16)        # block-diag W1^T (c-chunks)
    w2_d = din("w2K", [4, 128, OUT], f16)        # block-diag W2^T (m-chunks)
    bgcol_d = din("bgcol", [2, 128], f32)        # bg per o-tile (per-partition)
    b1col_d = din("b1col", [4, 128], f32)        # b1 per m-chunk (per-partition)
    g1col_d = din("g1col", [4, 128], f32)        # g1 per m-chunk (per-partition)
    b1_d = din("b1row", [1, MID], f16)
    b2_d = din("b2row", [1, OUT], f16)
    g2r16_d = din("g2row16", [1, OUT], f16)
    beta1col_d = din("beta1col", [4, 128], f32)
    beta2_d = din("beta2row", [1, OUT], f16)
    ident_d = din("ident", [128, 128], f16)
    ones_d = din("onescol", [1, 128], f16)

    gtsT_d = dout("gtsT", [B_LOC, OUT, N], f16)  # [o, n] - host un-transposes
    node_d = dout("node", [B_LOC, N, OUT], f16)
    out2_d = dout("out2", [B_LOC, N, OUT], f16)

    with tile.TileContext(nc) as tc:
        with tc.tile_pool(name="const", bufs=1) as constp, \
             tc.tile_pool(name="inp", bufs=1) as inp, \
             tc.tile_pool(name="at", bufs=1) as atp, \
             tc.tile_pool(name="big", bufs=1) as bigp, \
             tc.tile_pool(name="work", bufs=4) as workp, \
             tc.tile_pool(name="sg", bufs=3) as sgp, \
             tc.tile_pool(name="outs", bufs=1) as outp, \
             tc.tile_pool(name="mm", bufs=6, space="PSUM") as mmp, \
             tc.tile_pool(name="tp", bufs=2, space="PSUM") as tpp:

            # ---- f16 weights + x early on sync (tiny transfers, needed by
            # B/C within ~10us) ----
            ident_t = constp.tile([128, 128], f16)
            nc.sync.dma_start(out=ident_t, in_=ident_d)
            ones_t = constp.tile([1, 128], f16)
            nc.sync.dma_start(out=ones_t, in_=ones_d)
            wg_t = constp.tile([128, 2, OUT], f16)
            nc.sync.dma_start(out=wg_t, in_=wg_d.rearrange("c p f -> p c f"))
            w1_t = constp.tile([128, 2, MID], f16)
            nc.sync.dma_start(out=w1_t, in_=w1_d.rearrange("c p f -> p c f"))
            w2_t = constp.tile([128, 4, OUT], f16)
            nc.sync.dma_start(out=w2_t, in_=w2_d.rearrange("c p f -> p c f"))
            b1_t = constp.tile([1, MID], f16)
            nc.sync.dma_start(out=b1_t, in_=b1_d)
            b2_t = constp.tile([1, OUT], f16)
            nc.sync.dma_start(out=b2_t, in_=b2_d)
            xT_t = inp.tile([128, B_LOC, 2, N], f16)
            nc.sync.dma_start(
                out=xT_t, in_=xT_d.rearrange("b (c p) n -> p b c n", p=128)
            )

            gtT_t = inp.tile([128, B_LOC, 2, N], f16)
            nc.sync.dma_start(
                out=gtT_t, in_=gtT_d.rearrange("b (c p) n -> p b c n", p=128)
            )
            At = [
                [atp.tile([128, N], f16, name=f"At{b}_{jt}", tag=f"At{b}_{jt}")
                 for jt in range(NT)]
                for b in range(B_LOC)
            ]
            for b in range(B_LOC):
                for jt in range(NT):
                    nc.sync.dma_start(
                        out=At[b][jt], in_=msT_d[b, jt * 128 : (jt + 1) * 128, :]
                    )

            # ---- ACT queue: lirow/ljT first so sigmoids start immediately ----
            lirow_t = inp.tile([128, B_LOC, N], f16)
            nc.scalar.dma_start(
                out=lirow_t, in_=lirow_d[None].to_broadcast([128, B_LOC, N])
            )
            ljT_t = inp.tile([128, B_LOC, NT], f32)
            nc.scalar.dma_start(out=ljT_t, in_=ljT_d.rearrange("b p t -> p b t"))

            # ---- f32 / late-needed consts on gpsimd (idle until ~12us) ----
            bgcol_t = constp.tile([128, 2], f32)
            nc.gpsimd.dma_start(out=bgcol_t, in_=bgcol_d.rearrange("o p -> p o"))
            b1col_t = constp.tile([128, 4], f32)
            nc.gpsimd.dma_start(out=b1col_t, in_=b1col_d.rearrange("c p -> p c"))
            g1col_t = constp.tile([128, 4], f32)
            nc.gpsimd.dma_start(out=g1col_t, in_=g1col_d.rearrange("c p -> p c"))
            g2r16_t = constp.tile([128, OUT], f16)
            nc.gpsimd.dma_start(out=g2r16_t, in_=g2r16_d.to_broadcast([128, OUT]))
            if beta1_nz:
                beta1col_t = constp.tile([128, 4], f32)
                nc.gpsimd.dma_start(
                    out=beta1col_t, in_=beta1col_d.rearrange("c p -> p c")
                )
            if beta2_nz:
                beta2_t = constp.tile([128, OUT], f16)
                nc.gpsimd.dma_start(out=beta2_t, in_=beta2_d.to_broadcast([128, OUT]))
            eps_t = constp.tile([128, 1], f32)
            nc.vector.memset(eps_t, EPS_LN)

            # Per-batch activation tensors (both batches resident).
            o1t = [bigp.tile([128, NT, MID], f16, name=f"o1t{b}", tag=f"o1t{b}")
                   for b in range(B_LOC)]
            o1mn = [bigp.tile([128, 4, N], f16, name=f"o1mn{b}", tag=f"o1mn{b}")
                    for b in range(B_LOC)]
            zt = [bigp.tile([128, NT, MID], f16, name=f"zt{b}", tag=f"zt{b}")
                  for b in range(B_LOC)]
            o1n = [bigp.tile([128, 4, N], f16, name=f"o1n{b}", tag=f"o1n{b}")
                   for b in range(B_LOC)]
            o2t = [bigp.tile([128, NT, OUT], f16, name=f"o2t{b}", tag=f"o2t{b}")
                   for b in range(B_LOC)]

            gts_o = outp.tile([128, B_LOC, 2, N], f16)
            node_o = outp.tile([128, B_LOC, NT, OUT], f16)
            out2_o = outp.tile([128, B_LOC, NT, OUT], f16)

            def stage_A(b):
                for jt in range(NT):
                    sg = sgp.tile([128, N], f16, name="sg", tag="sg")
                    nc.scalar.activation(
                        out=sg, in_=lirow_t[:, b, :], func=AF.Sigmoid,
                        bias=ljT_t[:, b, jt : jt + 1], scale=1.0,
                    )
                    nc.vector.tensor_tensor(
                        out=At[b][jt], in0=At[b][jt], in1=sg, op=OP.mult
                    )

            def stage_C(b):
                # gconv1 -> o1t [j, m]
                for jt in range(NT):
                    ps = mmp.tile([128, MID], f32, name="ps", tag="ps")
                    nc.tensor.matmul(ps, lhsT=ones_t, rhs=b1_t, start=True, stop=False)
                    for cc in range(2):
                        nc.tensor.matmul(
                            ps,
                            lhsT=xT_t[:, b, cc, jt * 128 : (jt + 1) * 128],
                            rhs=w1_t[:, cc, :],
                            start=False, stop=(cc == 1),
                        )
                    nc.scalar.activation(out=o1t[b][:, jt, :], in_=ps, func=AF.Relu)
                # gconv1 -> o1mn [m, n] (bias per-partition)
                for mc in range(4):
                    for nh in range(2):
                        ps = mmp.tile([128, MID], f32, name="ps", tag="ps")
                        for cc in range(2):
                            nc.tensor.matmul(
                                ps,
                                lhsT=w1_t[:, cc, mc * 128 : (mc + 1) * 128],
                                rhs=xT_t[:, b, cc, nh * 512 : (nh + 1) * 512],
                                start=(cc == 0), stop=(cc == 1),
                            )
                        nc.scalar.activation(
                            out=o1mn[b][:, mc, nh * 512 : (nh + 1) * 512],
                            in_=ps, func=AF.Relu,
                            bias=b1col_t[:, mc : mc + 1], scale=1.0,
                        )

            def stage_B(b):
                # gts in [o, n] layout
                for ot in range(2):
                    for nh in range(2):
                        ps = mmp.tile([128, MID], f32, name="ps", tag="ps")
                        p5 = ps[:, :512]
                        for cc in range(2):
                            nc.tensor.matmul(
                                p5,
                                lhsT=wg_t[:, cc, ot * 128 : (ot + 1) * 128],
                                rhs=gtT_t[:, b, cc, nh * 512 : (nh + 1) * 512],
                                start=(cc == 0), stop=(cc == 1),
                            )
                        nc.scalar.activation(
                            out=gts_o[:, b, ot, nh * 512 : (nh + 1) * 512],
                            in_=p5, func=AF.Relu,
                            bias=bgcol_t[:, ot : ot + 1], scale=1.0,
                        )

            def stage_D(b):
                # o1m^T contraction + z = (o1m - mean)*rstd
                for w in range(2):  # waves of 4 i-tiles
                    its = [w * 4 + k for k in range(4)]
                    pss = [mmp.tile([128, MID], f32, name="psw", tag="ps")
                           for _ in its]
                    for jc in range(NT):
                        for k, it in enumerate(its):
                            nc.tensor.matmul(
                                pss[k],
                                lhsT=At[b][jc][:, it * 128 : (it + 1) * 128],
                                rhs=o1t[b][:, jc, :],
                                start=(jc == 0), stop=(jc == NT - 1),
                            )
                    mvw = workp.tile([128, 2, 4], f32, name="mvw", tag="mvw")
                    for k, it in enumerate(its):
                        sv = workp.tile([128, 6], f32, name="sv", tag="sv")
                        nc.vector.bn_stats(out=sv, in_=pss[k])
                        nc.vector.bn_aggr(out=mvw[:, :, k], in_=sv)
                    stdw = workp.tile([128, 4], f32, name="stdw", tag="stdw")
                    nc.scalar.activation(
                        out=stdw, in_=mvw[:, 1, :], func=AF.Sqrt, bias=eps_t
                    )
                    rstdw = workp.tile([128, 4], f32, name="rstdw", tag="rstdw")
                    nc.vector.reciprocal(out=rstdw, in_=stdw)
                    for k, it in enumerate(its):
                        nc.vector.tensor_scalar(
                            out=zt[b][:, it, :], in0=pss[k],
                            scalar1=mvw[:, 0, k : k + 1],
                            scalar2=rstdw[:, k : k + 1],
                            op0=OP.subtract, op1=OP.mult,
                        )

            def stage_E(b):
                # transpose z, fused *g1 + o1mn -> o1n [m, j]; gconv2 -> o2t
                for mc in range(4):
                    tp = tpp.tile([128, N], f16, name="tp", tag="tp")
                    for it in range(NT):
                        nc.tensor.transpose(
                            tp[:, it * 128 : (it + 1) * 128],
                            zt[b][:, it, mc * 128 : (mc + 1) * 128],
                            ident_t,
                        )
                    nc.vector.scalar_tensor_tensor(
                        out=o1n[b][:, mc, :], in0=tp,
                        scalar=g1col_t[:, mc : mc + 1],
                        in1=o1mn[b][:, mc, :], op0=OP.mult, op1=OP.add,
                    )
                    if beta1_nz:
                        nc.gpsimd.tensor_scalar_add(
                            o1n[b][:, mc, :], o1n[b][:, mc, :],
                            beta1col_t[:, mc : mc + 1],
                        )
                for jt in range(NT):
                    ps = mmp.tile([128, MID], f32, name="ps", tag="ps")
                    p256 = ps[:, :OUT]
                    nc.tensor.matmul(p256, lhsT=ones_t, rhs=b2_t, start=True, stop=False)
                    for mc in range(4):
                        nc.tensor.matmul(
                            p256,
                            lhsT=o1n[b][:, mc, jt * 128 : (jt + 1) * 128],
                            rhs=w2_t[:, mc, :],
                            start=False, stop=(mc == 3),
                        )
                    nc.vector.tensor_scalar_max(o2t[b][:, jt, :], p256, 0.0)

            def stage_F(b, eng):
                # o2m^T contraction + ln2 -> node_feat, output2.  The nf/out2
                # elementwise tail goes on `eng`: gpsimd for batch 0 (overlaps
                # batch 1 PE work), vector for batch 1 (shortest kernel tail).
                for w in range(2):
                    its = [w * 4 + k for k in range(4)]
                    pss = [mmp.tile([128, MID], f32, name="psw", tag="ps")
                           for _ in its]
                    for jc in range(NT):
                        for k, it in enumerate(its):
                            nc.tensor.matmul(
                                pss[k][:, :OUT],
                                lhsT=At[b][jc][:, it * 128 : (it + 1) * 128],
                                rhs=o2t[b][:, jc, :],
                                start=(jc == 0), stop=(jc == NT - 1),
                            )
                    mvw = workp.tile([128, 2, 4], f32, name="mvw", tag="mvw")
                    for k, it in enumerate(its):
                        sv = workp.tile([128, 6], f32, name="sv", tag="sv")
                        nc.vector.bn_stats(out=sv, in_=pss[k][:, :OUT])
                        nc.vector.bn_aggr(out=mvw[:, :, k], in_=sv)
                    stdw = workp.tile([128, 4], f32, name="stdw", tag="stdw")
                    nc.scalar.activation(
                        out=stdw, in_=mvw[:, 1, :], func=AF.Sqrt, bias=eps_t
                    )
                    rstdw = workp.tile([128, 4], f32, name="rstdw", tag="rstdw")
                    nc.vector.reciprocal(out=rstdw, in_=stdw)
                    for k, it in enumerate(its):
                        z2 = workp.tile([128, OUT], f16, name="z2", tag="z2")
                        nc.vector.tensor_scalar(
                            out=z2, in0=pss[k][:, :OUT],
                            scalar1=mvw[:, 0, k : k + 1],
                            scalar2=rstdw[:, k : k + 1],
                            op0=OP.subtract, op1=OP.mult,
                        )
                        nf = node_o[:, b, it, :]
                        eng.tensor_tensor(out=nf, in0=z2, in1=g2r16_t, op=OP.mult)
                        if beta2_nz:
                            eng.tensor_tensor(out=nf, in0=nf, in1=beta2_t, op=OP.add)
                        eng.tensor_tensor(
                            out=out2_o[:, b, it, :], in0=nf,
                            in1=o2t[b][:, it, :], op=OP.add,
                        )

            def dma_gts(b):
                nc.gpsimd.dma_start(
                    out=gtsT_d[b].rearrange("(o p) n -> p o n", p=128),
                    in_=gts_o[:, b],
                )

            def dma_nodeout2(b):
                nc.gpsimd.dma_start(
                    out=node_d[b].rearrange("(t p) o -> p t o", p=128),
                    in_=node_o[:, b],
                )
                nc.gpsimd.dma_start(
                    out=out2_d[b].rearrange("(t p) o -> p t o", p=128),
                    in_=out2_o[:, b],
                )

            stage_A(0)
            stage_C(0)
            stage_B(0)
            stage_D(0)
            stage_A(1)
            stage_C(1)
            stage_B(1)
            dma_gts(0)
            dma_gts(1)
            stage_E(0)
            stage_F(0, nc.gpsimd)
            dma_nodeout2(0)
            stage_D(1)
            stage_E(1)
            stage_F(1, nc.vector)
            dma_nodeout2(1)

    nc.compile()
    return nc


def _compute_col_fast(m1, m2, sm):
    """Exact col == ones proof via a cheap sufficient condition, else None."""
    if m1.min() < 0.0 or m2.min() < 0.0 or sm.min() < 0.0:
        return None
    spos = (sm > 0).astype(F32)
    colnz = np.zeros(N, dtype=bool)
    nz1max = 0.0
    nz2max = 0.0
    for b in range(B):
        p1 = (m1[b] > 0).astype(F32)
        p2 = (m2[b] > 0).astype(F32)
        nz1max = max(nz1max, float((p1 @ spos[b]).max()))
        nz2max = max(nz2max, float((p2 @ spos[b]).max()))
        colnz |= ((p1 + p2).max(axis=0) > 0) & (spos[b] > 0)
    if nz1max <= CHILDS // 4 and nz2max <= CHILDS // 2 and colnz.all():
        return np.ones(N, dtype=F32)
    return None


def _compute_col_slow(m1, m2, sm, li, lj):
    """Exact replica of the reference top-k column-union (numpy)."""
    k4, k2 = CHILDS // 4, CHILDS // 2
    col = np.zeros(N, dtype=bool)
    for b in range(B):
        logits = li[b][:, None] + lj[b][None, :]
        a = 1.0 / (1.0 + np.exp(-logits.astype(F32)))
        mr1 = m1[b] * sm[b][None, :]
        mr2 = m2[b] * sm[b][None, :]
        a1 = a * mr1
        a2 = a * mr2
        # lax.top_k ties -> lowest index; stable argsort on (-a) reproduces it.
        col[np.argsort(-a1, axis=1, kind="stable")[:, :k4].ravel()] = True
        col[np.argsort(a1, axis=1, kind="stable")[:, :k4].ravel()] = True
        col[np.argsort(-a2, axis=1, kind="stable")[:, :k2].ravel()] = True
        col[np.argsort(a2, axis=1, kind="stable")[:, :k4].ravel()] = True
    return col.astype(F32)


def kernel(**inputs):
    x = np.ascontiguousarray(np.asarray(inputs["x"], dtype=F32))
    m1 = np.asarray(inputs["masks_roi1"], dtype=F32)
    m2 = np.asarray(inputs["masks_roi2"], dtype=F32)
    sm = np.asarray(inputs["score_mask"], dtype=F32)
    gt = np.asarray(inputs["gt_feat"], dtype=F32)
    W_att = np.asarray(inputs["W_att"], dtype=F32)
    b_att = np.asarray(inputs["b_att"], dtype=F32)
    W1 = np.asarray(inputs["W1"], dtype=F32)
    b1 = np.asarray(inputs["b1"], dtype=F32)
    W2 = np.asarray(inputs["W2"], dtype=F32)
    b2 = np.asarray(inputs["b2"], dtype=F32)
    g1 = np.asarray(inputs["g1"], dtype=F32)
    beta1 = np.asarray(inputs["beta1"], dtype=F32)
    g2 = np.asarray(inputs["g2"], dtype=F32)
    beta2 = np.asarray(inputs["beta2"], dtype=F32)
    Wg = np.asarray(inputs["Wg"], dtype=F32)
    bg = np.asarray(inputs["bg"], dtype=F32)

    assert x.shape == (B, N, CIN) and W_att.shape == (2 * CIN, 1)

    # ---- host prep: tiny vector math + layout/dtype staging ----
    lj = x.reshape(B * N, CIN) @ W_att[:CIN, 0]
    lj = lj.reshape(B, N)
    li = x.reshape(B * N, CIN) @ W_att[CIN:, 0]
    li = li.reshape(B, N) + b_att[0]

    col = _compute_col_fast(m1, m2, sm)
    if col is None:
        col = _compute_col_slow(m1, m2, sm, li, lj)

    # One fused mask tensor: msT[j, i] = (m1+m2)[i, j] * score[j] * col[j],
    # with the f_diag term folded exactly onto the diagonal as
    # f[j] / sigmoid(l_jj)   (then the device's sigmoid multiply restores f).
    msT = (m1 + m2).transpose(0, 2, 1) * (sm * col[None, :])[:, :, None]
    f = (sm == 0).astype(F32)
    ldiag = li + lj  # l_jj = li[j] + lj[j]
    sig_diag = 1.0 / (1.0 + np.exp(-ldiag))
    didx = np.arange(N)
    msT[:, didx, didx] += f / sig_diag
    msT = np.ascontiguousarray(msT).astype(F16)

    xT = np.ascontiguousarray(x.transpose(0, 2, 1)).astype(F16)
    gtT = np.ascontiguousarray(gt.transpose(0, 2, 1)).astype(F16)
    lirow = li.astype(F16)
    ljT = np.ascontiguousarray(lj.reshape(B, NT, 128).transpose(0, 2, 1)).astype(F32)

    # Weights: block-diagonal transposed layouts for the grouped convs.
    w1bd = np.zeros((CIN, MID), dtype=F32)
    for g in range(G):
        w1bd[64 * g : 64 * (g + 1), 128 * g : 128 * (g + 1)] = W1[
            128 * g : 128 * (g + 1), :
        ].T
    w2bd = np.zeros((MID, OUT), dtype=F32)
    for g in range(G):
        w2bd[128 * g : 128 * (g + 1), 64 * g : 64 * (g + 1)] = W2[
            64 * g : 64 * (g + 1), :
        ].T
    wgK = np.ascontiguousarray(Wg.reshape(2, 128, OUT)).astype(F16)
    w1K = np.ascontiguousarray(w1bd.reshape(2, 128, MID)).astype(F16)
    w2K = np.ascontiguousarray(w2bd.reshape(4, 128, OUT)).astype(F16)

    shared = {
        "wgK": wgK,
        "w1K": w1K,
        "w2K": w2K,
        "bgcol": np.ascontiguousarray(bg.reshape(2, 128)).astype(F32),
        "b1col": np.ascontiguousarray(b1.reshape(4, 128)).astype(F32),
        "g1col": np.ascontiguousarray(g1.reshape(4, 128)).astype(F32),
        "b1row": b1.reshape(1, MID).astype(F16),
        "b2row": b2.reshape(1, OUT).astype(F16),
        "g2row16": g2.reshape(1, OUT).astype(F16),
        "beta1col": np.ascontiguousarray(beta1.reshape(4, 128)).astype(F32),
        "beta2row": beta2.reshape(1, OUT).astype(F16),
        "ident": np.eye(128, dtype=F16),
        "onescol": np.ones((1, 128), dtype=F16),
    }
    in_maps = []
    for c in range(NCORES):
        s = slice(B_LOC * c, B_LOC * (c + 1))
        in_maps.append(
            {
                "msT": msT[s],
                "xT": xT[s],
                "gtT": gtT[s],
                "lirow": lirow[s],
                "ljT": ljT[s],
                **shared,
            }
        )

    beta_key = (bool(np.any(beta1)), bool(np.any(beta2)))
    if beta_key not in _PROGRAM_CACHE:
        _PROGRAM_CACHE[beta_key] = _build_program(*beta_key)
    nc = _PROGRAM_CACHE[beta_key]

    global _LAST_IN_MAPS
    _LAST_IN_MAPS = in_maps

    from concourse.bass_utils import run_bass_kernel_spmd

    res = run_bass_kernel_spmd(nc, in_maps, core_ids=list(range(NCORES)))
    results = res.results if hasattr(res, "results") else res

    gts = np.concatenate(
        [results[c]["gtsT"].transpose(0, 2, 1) for c in range(NCORES)], axis=0
    )
    node_feat = np.concatenate([results[c]["node"] for c in range(NCORES)], axis=0)
    output2 = np.concatenate([results[c]["out2"] for c in range(NCORES)], axis=0)
    return output2.astype(F32), gts.astype(F32), node_feat.astype(F32)
